# revision 1
# baseline (speedup 1.0000x reference)
"""Trainium2 Bass kernel for gathered-row MLP decode matmul.

out[b, 0, r] = sum_d x[b, 0, d] * weight[indices[r], d]

Strategy: dedup+sort the indices on the host, shard them contiguously
across 8 cores, pad per-core to a fixed multiple of 128. The fp32 weight is
split on the host into an fp16 hi/lo pair (residual pre-scaled by 2^11 to
stay in fp16 normal range; hi + lo/2048 reconstructs fp32 to ~2^-22).
Each core gathers its rows of both halves with dma_gather(transpose=True),
which lands them directly in matmul-ready [d%128, d//128, r] layout — no
on-chip transposes. The x operand is pre-transposed and hi/lo-split on the
host and packed [xh||xl] so one M=64 matmul computes xh*whi and xl*whi in a
single moving pass; a second M=32 matmul adds xh*wlo. The two lo-products
are summed on partitions 32-63, realigned to 0-31 with a small SBUF-SBUF
DMA, scaled by 2^-11 and added to the hi chain — fp32-class accuracy with
2 moving passes per contraction tile. Host scatters the per-core unique
outputs back to the original 4403 index order.
"""

import os
import sys
from contextlib import ExitStack

sys.path.insert(0, "/opt/trn_rl_repo")
os.environ.setdefault("MYCRO_LOCAL_CACHE", "1")

import numpy as np

D_FF = 11008
D_MODEL = 4096
R_TOTAL = 4403
B = 32
NCORES = 8
P = 128
KT = D_MODEL // P          # 32 contraction tiles
NPAD = 640                 # padded per-core index count (5*128), fallback
NPAD_DEDUP = 512           # padded per-core count for the dedup path
LO_SCALE = 2048.0          # wlo/xlo pre-scale (2^11)

# per-core share of the real 4403 indices (no-dedup fallback)
_CORE_N = [551, 551, 551, 550, 550, 550, 550, 550]
_CORE_START = [0]
for _n in _CORE_N[:-1]:
    _CORE_START.append(_CORE_START[-1] + _n)

_cache = {}


def _build(
    reps=1, mode="full", tiny_out=False, npad=NPAD, chunks=None, gbufs=2,
    nvalid=None,
):
    """mode: full (3-matmul) | fused (2-pass M=64 packing) | dma (gathers
    only) | dma_nt (non-transpose gathers) | mm (matmuls only).
    tiny_out: shrink the DRAM output to [B, 64] so bench-loop host
    transfers are negligible (timing only)."""
    if nvalid is None:
        nvalid = npad
    key = ("nc", reps, mode, tiny_out, npad, chunks, gbufs, nvalid)
    if key in _cache:
        return _cache[key]
    from concourse import bacc, mybir, tile

    f32 = mybir.dt.float32
    f16 = mybir.dt.float16
    i16 = mybir.dt.int16

    if chunks is None:
        chunks = tuple((i, min(256, npad - i)) for i in range(0, npad, 256))

    nc = bacc.Bacc(
        "TRN2", target_bir_lowering=False, debug=False, enable_asserts=False
    )
    whi_dram = nc.dram_tensor("whi", [D_FF, D_MODEL], f16, kind="ExternalInput").ap()
    wlo_dram = nc.dram_tensor("wlo", [D_FF, D_MODEL], f16, kind="ExternalInput").ap()
    if mode in ("fused", "fused2"):
        xp_dram = nc.dram_tensor("xp", [P, KT * 2 * B], f16, kind="ExternalInput").ap()
        if mode == "fused2":
            combm_dram = nc.dram_tensor(
                "combm", [P, B], f32, kind="ExternalInput"
            ).ap()
    else:
        xh_dram = nc.dram_tensor("xh", [P, KT * B], f16, kind="ExternalInput").ap()
        xl_dram = nc.dram_tensor("xl", [P, KT * B], f16, kind="ExternalInput").ap()
    idx_dram = nc.dram_tensor("idx", [P, npad // 16], i16, kind="ExternalInput").ap()
    out_cols = 64 if tiny_out else npad
    out_dram = nc.dram_tensor("out", [B, out_cols], f32, kind="ExternalOutput").ap()

    with tile.TileContext(nc) as tc, ExitStack() as ctx:
        consts = ctx.enter_context(tc.tile_pool(name="consts", bufs=1))
        whi_pool = ctx.enter_context(tc.tile_pool(name="whiT", bufs=gbufs))
        wlo_pool = ctx.enter_context(tc.tile_pool(name="wloT", bufs=gbufs))
        psum = ctx.enter_context(tc.tile_pool(name="psum", bufs=4, space="PSUM"))
        out_pool = ctx.enter_context(tc.tile_pool(name="outp", bufs=2))

        # idx first: the gathers (the critical path) depend only on it
        idx_sb = consts.tile([P, npad // 16], i16)
        nc.sync.dma_start(idx_sb[:], idx_dram)
        if mode in ("fused", "fused2"):
            xp_sb = consts.tile([P, KT * 2 * B], f16)
            nc.sync.dma_start(xp_sb[:], xp_dram)
        else:
            xh_sb = consts.tile([P, KT * B], f16)
            nc.sync.dma_start(xh_sb[:], xh_dram)
            xl_sb = consts.tile([P, KT * B], f16)
            nc.sync.dma_start(xl_sb[:], xl_dram)

        if mode == "mm":
            whiT_c = consts.tile([P, KT, 256], f16)
            nc.gpsimd.memset(whiT_c[:], 0.25)
            wloT_c = consts.tile([P, KT, 256], f16)
            nc.gpsimd.memset(wloT_c[:], 0.25)

        if mode == "fused2":
            # combM.T @ [hi; lo; 0] = hi + lo/2048 (host-built constant;
            # K padded to 128 so the fp32 matmul uses the proven full
            # partition-group shape)
            combM = consts.tile([P, B], f32)
            nc.sync.dma_start(combM[:], combm_dram)

        for _rep in range(reps):
            out_sb = out_pool.tile([B, npad], f32, tag="out_sb")
            if mode == "fused":
                t1_sb = out_pool.tile([64, npad], f32, tag="t1")
                outA_sb = out_pool.tile([B, npad], f32, tag="outA")

            for c, (r0, ncols) in enumerate(chunks):
                if mode in ("full", "fused", "fused2", "dma"):
                    # valid (non -1) indices in this chunk; the SWDGE trims
                    # the transfer to the valid prefix, so -1 tail padding
                    # costs no DMA.
                    nval_c = max(0, min(nvalid - r0, ncols))
                    # whiT[p, k, i] = whi[idx[r0+i], k*128 + p]
                    whiT = whi_pool.tile([P, KT, ncols], f16, tag="whiT")
                    wloT = wlo_pool.tile([P, KT, ncols], f16, tag="wloT")
                    if _rep == 0 and nval_c < ncols:
                        # first use of the slot: zero the never-gathered tail
                        # columns so downstream matmuls read finite data
                        nc.vector.memset(whiT[:, :, nval_c:], 0)
                        nc.vector.memset(wloT[:, :, nval_c:], 0)
                    nc.gpsimd.dma_gather(
                        out_ap=whiT[:],
                        in_ap=whi_dram,
                        idxs_ap=idx_sb[:, r0 // 16 : (r0 + ncols) // 16],
                        num_idxs=ncols,
                        num_idxs_reg=nval_c,
                        elem_size=D_MODEL,
                        transpose=True,
                    )
                    nc.gpsimd.dma_gather(
                        out_ap=wloT[:],
                        in_ap=wlo_dram,
                        idxs_ap=idx_sb[:, r0 // 16 : (r0 + ncols) // 16],
                        num_idxs=ncols,
                        num_idxs_reg=nval_c,
                        elem_size=D_MODEL,
                        transpose=True,
                    )
                elif mode == "dma_nt":
                    whiT = whi_pool.tile([P, -(-ncols // P), D_MODEL], f16, tag="whiT")
                    nc.gpsimd.dma_gather(
                        out_ap=whiT[:],
                        in_ap=whi_dram,
                        idxs_ap=idx_sb[:, r0 // 16 : (r0 + ncols) // 16],
                        num_idxs=ncols,
                        num_idxs_reg=ncols,
                        elem_size=D_MODEL,
                        transpose=False,
                    )
                    wloT = wlo_pool.tile([P, -(-ncols // P), D_MODEL], f16, tag="wloT")
                    nc.gpsimd.dma_gather(
                        out_ap=wloT[:],
                        in_ap=wlo_dram,
                        idxs_ap=idx_sb[:, r0 // 16 : (r0 + ncols) // 16],
                        num_idxs=ncols,
                        num_idxs_reg=ncols,
                        elem_size=D_MODEL,
                        transpose=False,
                    )
                else:
                    whiT = whiT_c
                    wloT = wloT_c

                if mode in ("dma", "dma_nt"):
                    continue

                if mode in ("fused", "fused2"):
                    # One PSUM chain: rows 0-31 accumulate xh*whi (hi chain);
                    # rows 32-63 accumulate xl_s*whi (mm1) AND xh*wlo_s (mm2).
                    # The group is opened by mm1@k=0 (spans rows 0-63) and
                    # closed by mm1@k=31, so mm2@k=31 is emitted before it.
                    psAB = psum.tile([64, ncols], mybir.dt.float32, tag="psA")

                    def mm1(k):
                        nc.tensor.matmul(
                            out=psAB[:],
                            lhsT=xp_sb[:, k * 2 * B : (k + 1) * 2 * B],
                            rhs=whiT[:, k, :],
                            start=(k == 0),
                            stop=(k == KT - 1),
                        )

                    def mm2(k):
                        nc.tensor.matmul(
                            out=psAB[B : 2 * B, :],
                            lhsT=xp_sb[:, k * 2 * B : k * 2 * B + B],
                            rhs=wloT[:, k, :],
                            start=False,
                            stop=False,
                        )

                    # mm1s first: they only depend on the whi gather, so the
                    # PE starts before wlo lands. mm1@KT-1 closes the group.
                    for k in range(KT - 1):
                        mm1(k)
                    for k in range(KT):
                        mm2(k)
                    mm1(KT - 1)
                    if mode == "fused2":
                        # recombine on the PE: out = combM.T @ [hi; lo; 0]
                        # = hi + lo/2048, landing directly on partitions 0-31
                        cmb_sb = out_pool.tile([P, ncols], f32, tag="cmb")
                        if _rep == 0 and c < 2:
                            nc.vector.memset(cmb_sb[2 * B :, :], 0)
                        nc.vector.tensor_copy(cmb_sb[: 2 * B, :], psAB[:])
                        psO = psum.tile([B, ncols], mybir.dt.float32, tag="psO")
                        nc.tensor.matmul(
                            out=psO[:], lhsT=combM[:], rhs=cmb_sb[:],
                            start=True, stop=True,
                        )
                        nc.scalar.copy(out_sb[:, r0 : r0 + ncols], psO[:])
                        if not tiny_out:
                            nc.sync.dma_start(
                                out_dram[:, r0 : r0 + ncols],
                                out_sb[:, r0 : r0 + ncols],
                            )
                        continue
                    # hi chain copied to partitions 0-31; lo-sum scaled on
                    # 32-63, realigned to 0-31 with an HWDGE SBUF-SBUF DMA
                    # (keeps the tail off the SWDGE queue the gathers use),
                    # then added and written out per chunk.
                    nc.scalar.copy(outA_sb[:, r0 : r0 + ncols], psAB[:B, :])
                    nc.vector.tensor_scalar_mul(
                        t1_sb[B : 2 * B, r0 : r0 + ncols],
                        psAB[B : 2 * B, :],
                        1.0 / LO_SCALE,
                    )
                    nc.sync.dma_start(
                        out_sb[:, r0 : r0 + ncols],
                        t1_sb[B : 2 * B, r0 : r0 + ncols],
                    )
                    nc.vector.tensor_add(
                        out_sb[:, r0 : r0 + ncols],
                        out_sb[:, r0 : r0 + ncols],
                        outA_sb[:, r0 : r0 + ncols],
                    )
                    if not tiny_out:
                        nc.sync.dma_start(
                            out_dram[:, r0 : r0 + ncols],
                            out_sb[:, r0 : r0 + ncols],
                        )
                    continue

                # mode full/mm: 3 matmul passes, both chains on partitions 0-31
                mcols = 256 if mode == "mm" else ncols
                psA = psum.tile([B, mcols], mybir.dt.float32, tag="psA")
                psB = psum.tile([B, mcols], mybir.dt.float32, tag="psB")
                for k in range(KT):
                    xh_k = xh_sb[:, k * B : (k + 1) * B]
                    xl_k = xl_sb[:, k * B : (k + 1) * B]
                    nc.tensor.matmul(
                        out=psA[:],
                        lhsT=xh_k,
                        rhs=whiT[:, k, :mcols],
                        start=(k == 0),
                        stop=(k == KT - 1),
                    )
                    nc.tensor.matmul(
                        out=psB[:],
                        lhsT=xh_k,
                        rhs=wloT[:, k, :mcols],
                        start=(k == 0),
                        stop=False,
                    )
                    nc.tensor.matmul(
                        out=psB[:],
                        lhsT=xl_k,
                        rhs=whiT[:, k, :mcols],
                        start=False,
                        stop=(k == KT - 1),
                    )
                dst = out_sb[:, r0 : r0 + ncols]
                nc.scalar.mul(dst, psB[:, :ncols], 1.0 / LO_SCALE)
                nc.vector.tensor_add(dst, dst, psA[:, :ncols])

            if mode == "fused":
                if tiny_out:
                    nc.sync.dma_start(out_dram, outA_sb[:, :out_cols])
                continue
            if mode == "fused2":
                continue
            if mode in ("dma", "dma_nt"):
                nc.vector.tensor_copy(out_sb[:, :64], whiT[:32, 0, :64])
            nc.sync.dma_start(out_dram, out_sb[:, :out_cols])

    nc.compile()
    _cache[key] = nc
    return nc


def _split_pair(a):
    """fp32 -> (hi fp16, (a-hi)*2048 fp16). hi + lo/2048 ~= a to ~2^-22 rel."""
    hi = a.astype(np.float16)
    lo = ((a - hi.astype(np.float32)) * LO_SCALE).astype(np.float16)
    return hi, lo


def _wrap_idx(idx_pad):
    """[npad] int16 -> [128, npad//16] wrapped-16 layout, replicated 8x."""
    npad = idx_pad.shape[0]
    blk = idx_pad.reshape(npad // 16, 16).T  # [16, npad//16]
    return np.ascontiguousarray(np.tile(blk, (8, 1)))


def _make_in_maps(x, weight, indices, dedup=True):
    """Returns (in_maps, assemble_fn, npad)."""
    x = np.asarray(x, dtype=np.float32)
    weight = np.ascontiguousarray(np.asarray(weight, dtype=np.float32))
    indices = np.asarray(indices, dtype=np.int64)

    whi, wlo = _split_pair(weight)
    whi = np.ascontiguousarray(whi)
    wlo = np.ascontiguousarray(wlo)

    # x^T staged so the DMA is contiguous: xt[p, k*32+b] = x[b, 0, k*128+p]
    xt = np.ascontiguousarray(
        x[:, 0, :].reshape(B, KT, P).transpose(2, 1, 0).reshape(P, KT * B)
    )
    xh, xl = _split_pair(xt)
    # packed [xh || xl] per contraction tile for the fused M=64 matmul
    xp = np.empty((P, KT, 2 * B), dtype=np.float16)
    xp[:, :, :B] = xh.reshape(P, KT, B)
    xp[:, :, B:] = xl.reshape(P, KT, B)
    xp = np.ascontiguousarray(xp.reshape(P, KT * 2 * B))
    combm = np.zeros((P, B), dtype=np.float32)
    combm[:B, :] = np.eye(B, dtype=np.float32)
    combm[B : 2 * B, :] = np.eye(B, dtype=np.float32) / LO_SCALE

    uniq, inv = np.unique(indices, return_inverse=True)
    nu = len(uniq)
    use_dedup = dedup and -(-nu // NCORES) <= NPAD_DEDUP
    if use_dedup:
        npad = NPAD_DEDUP
        base, rem = divmod(nu, NCORES)
        counts = [base + (1 if c < rem else 0) for c in range(NCORES)]
        starts = np.concatenate([[0], np.cumsum(counts)[:-1]])
        core_idx = [uniq[starts[c] : starts[c] + counts[c]] for c in range(NCORES)]
    else:
        npad = NPAD
        counts = list(_CORE_N)
        starts = list(_CORE_START)
        core_idx = [
            indices[starts[c] : starts[c] + counts[c]] for c in range(NCORES)
        ]

    # uniform valid count (dup-padded with row 0); -1 beyond it is trimmed
    # from the gather transfer by the SWDGE
    nvalid = min(-(-max(counts) // 16) * 16, npad)

    in_maps = []
    for c in range(NCORES):
        idx_pad = np.full(npad, -1, dtype=np.int16)
        idx_pad[: counts[c]] = core_idx[c]
        idx_pad[counts[c] : nvalid] = 0
        in_maps.append(
            {
                "whi": whi,
                "wlo": wlo,
                "xh": np.ascontiguousarray(xh),
                "xl": np.ascontiguousarray(xl),
                "xp": xp,
                "combm": combm,
                "idx": _wrap_idx(idx_pad),
            }
        )

    def assemble(results):
        cols = np.empty((B, sum(counts)), dtype=np.float32)
        for c in range(NCORES):
            cols[:, starts[c] : starts[c] + counts[c]] = results[c]["out"][
                :, : counts[c]
            ]
        if use_dedup:
            out = cols[:, inv]
        else:
            out = cols
        return np.ascontiguousarray(out.reshape(B, 1, R_TOTAL))

    return in_maps, assemble, npad, nvalid


def _filter_in_maps(nc, in_maps):
    names = set()
    from concourse import mybir

    for alloc in nc.m.functions[0].allocations:
        if isinstance(alloc, mybir.MemoryLocationSet) and alloc.kind == "ExternalInput":
            names.add(alloc.memorylocations[0].name)
    return [{k: v for k, v in m.items() if k in names} for m in in_maps]


def run_full(x, weight, indices, trace=False, mode="fused2", dedup=True):
    """Returns (output, BassKernelResults)."""
    from concourse.bass_utils import run_bass_kernel_spmd

    in_maps, assemble, npad, nvalid = _make_in_maps(x, weight, indices, dedup=dedup)
    if mode == "fused2" and npad != NPAD_DEDUP:
        # the fused2 epilogue is only validated for the 512-wide dedup
        # layout; the rare >4096-unique fallback uses the fused tail
        mode = "fused"
    nc = _build(1, mode, False, npad, nvalid=nvalid)
    in_maps = _filter_in_maps(nc, in_maps)
    res = run_bass_kernel_spmd(nc, in_maps, list(range(NCORES)), trace=trace)
    return assemble(res.results), res


def kernel(x, weight, indices):
    out, _ = run_full(x, weight, indices)
    return out



# revision 14
# speedup vs baseline: 1.7534x; 1.7534x over previous
"""Trainium2 Bass kernel for gathered-row MLP decode matmul.

out[b, 0, r] = sum_d x[b, 0, d] * weight[indices[r], d]

Strategy: dedup+sort the indices on the host, shard them contiguously
across 8 cores, pad per-core to a fixed multiple of 128. The fp32 weight is
split on the host into an fp16 hi/lo pair (residual pre-scaled by 2^11 to
stay in fp16 normal range; hi + lo/2048 reconstructs fp32 to ~2^-22).
Each core gathers its rows of both halves with dma_gather(transpose=True),
which lands them directly in matmul-ready [d%128, d//128, r] layout — no
on-chip transposes. The x operand is pre-transposed and hi/lo-split on the
host and packed [xh||xl] so one M=64 matmul computes xh*whi and xl*whi in a
single moving pass; a second M=32 matmul adds xh*wlo. The two lo-products
are summed on partitions 32-63, realigned to 0-31 with a small SBUF-SBUF
DMA, scaled by 2^-11 and added to the hi chain — fp32-class accuracy with
2 moving passes per contraction tile. Host scatters the per-core unique
outputs back to the original 4403 index order.
"""

import os
import sys
from contextlib import ExitStack

sys.path.insert(0, "/opt/trn_rl_repo")
os.environ.setdefault("MYCRO_LOCAL_CACHE", "1")

import numpy as np

D_FF = 11008
D_MODEL = 4096
R_TOTAL = 4403
B = 32
NCORES = 8
P = 128
KT = D_MODEL // P          # 32 contraction tiles
NPAD = 640                 # padded per-core index count (5*128), fallback
NPAD_DEDUP = 512           # padded per-core count for the dedup path
LO_SCALE = 2048.0          # wlo/xlo pre-scale (2^11)

# per-core share of the real 4403 indices (no-dedup fallback)
_CORE_N = [551, 551, 551, 550, 550, 550, 550, 550]
_CORE_START = [0]
for _n in _CORE_N[:-1]:
    _CORE_START.append(_CORE_START[-1] + _n)

_cache = {}


def _build(
    reps=1, mode="full", tiny_out=False, npad=NPAD, chunks=None, gbufs=2,
    nvalid=None,
):
    """mode: full (3-matmul) | fused (2-pass M=64 packing) | dma (gathers
    only) | dma_nt (non-transpose gathers) | mm (matmuls only).
    tiny_out: shrink the DRAM output to [B, 64] so bench-loop host
    transfers are negligible (timing only)."""
    if nvalid is None:
        nvalid = npad
    key = ("nc", reps, mode, tiny_out, npad, chunks, gbufs, nvalid)
    if key in _cache:
        return _cache[key]
    from concourse import bacc, mybir, tile

    f32 = mybir.dt.float32
    f16 = mybir.dt.float16
    i16 = mybir.dt.int16

    if chunks is None:
        chunks = tuple((i, min(256, npad - i)) for i in range(0, npad, 256))

    nc = bacc.Bacc(
        "TRN2", target_bir_lowering=False, debug=False, enable_asserts=False
    )
    whi_dram = nc.dram_tensor("whi", [D_FF, D_MODEL], f16, kind="ExternalInput").ap()
    wlo_dram = nc.dram_tensor("wlo", [D_FF, D_MODEL], f16, kind="ExternalInput").ap()
    if mode in ("fused", "fused2"):
        xp_dram = nc.dram_tensor("xp", [P, KT * 2 * B], f16, kind="ExternalInput").ap()
        if mode == "fused2":
            combm_dram = nc.dram_tensor(
                "combm", [P, B], f32, kind="ExternalInput"
            ).ap()
    else:
        xh_dram = nc.dram_tensor("xh", [P, KT * B], f16, kind="ExternalInput").ap()
        xl_dram = nc.dram_tensor("xl", [P, KT * B], f16, kind="ExternalInput").ap()
    idx_dram = nc.dram_tensor("idx", [P, npad // 16], i16, kind="ExternalInput").ap()
    out_cols = 64 if tiny_out else npad
    out_dram = nc.dram_tensor("out", [B, out_cols], f32, kind="ExternalOutput").ap()

    with tile.TileContext(nc) as tc, ExitStack() as ctx:
        consts = ctx.enter_context(tc.tile_pool(name="consts", bufs=1))
        whi_pool = ctx.enter_context(tc.tile_pool(name="whiT", bufs=gbufs))
        wlo_pool = ctx.enter_context(tc.tile_pool(name="wloT", bufs=gbufs))
        psum = ctx.enter_context(tc.tile_pool(name="psum", bufs=4, space="PSUM"))
        out_pool = ctx.enter_context(tc.tile_pool(name="outp", bufs=2))

        # idx first: the gathers (the critical path) depend only on it
        idx_sb = consts.tile([P, npad // 16], i16)
        nc.sync.dma_start(idx_sb[:], idx_dram)
        if mode in ("fused", "fused2"):
            xp_sb = consts.tile([P, KT * 2 * B], f16)
            nc.sync.dma_start(xp_sb[:], xp_dram)
        else:
            xh_sb = consts.tile([P, KT * B], f16)
            nc.sync.dma_start(xh_sb[:], xh_dram)
            xl_sb = consts.tile([P, KT * B], f16)
            nc.sync.dma_start(xl_sb[:], xl_dram)

        if mode == "mm":
            whiT_c = consts.tile([P, KT, 256], f16)
            nc.gpsimd.memset(whiT_c[:], 0.25)
            wloT_c = consts.tile([P, KT, 256], f16)
            nc.gpsimd.memset(wloT_c[:], 0.25)

        if mode == "fused2":
            # combM.T @ [hi; lo; 0] = hi + lo/2048 (host-built constant;
            # K padded to 128 so the fp32 matmul uses the proven full
            # partition-group shape)
            combM = consts.tile([P, B], f32)
            nc.sync.dma_start(combM[:], combm_dram)

        for _rep in range(reps):
            out_sb = out_pool.tile([B, npad], f32, tag="out_sb")
            if mode == "fused":
                t1_sb = out_pool.tile([64, npad], f32, tag="t1")
                outA_sb = out_pool.tile([B, npad], f32, tag="outA")

            for c, (r0, ncols) in enumerate(chunks):
                if mode in ("full", "fused", "fused2", "dma"):
                    # valid (non -1) indices in this chunk; the SWDGE trims
                    # the transfer to the valid prefix, so -1 tail padding
                    # costs no DMA.
                    nval_c = max(0, min(nvalid - r0, ncols))
                    # whiT[p, k, i] = whi[idx[r0+i], k*128 + p]
                    whiT = whi_pool.tile([P, KT, ncols], f16, tag="whiT")
                    wloT = wlo_pool.tile([P, KT, ncols], f16, tag="wloT")
                    if _rep == 0 and nval_c < ncols:
                        # first use of the slot: zero the never-gathered tail
                        # columns so downstream matmuls read finite data
                        nc.vector.memset(whiT[:, :, nval_c:], 0)
                        nc.vector.memset(wloT[:, :, nval_c:], 0)
                    nc.gpsimd.dma_gather(
                        out_ap=whiT[:],
                        in_ap=whi_dram,
                        idxs_ap=idx_sb[:, r0 // 16 : (r0 + ncols) // 16],
                        num_idxs=ncols,
                        num_idxs_reg=nval_c,
                        elem_size=D_MODEL,
                        transpose=True,
                    )
                    nc.gpsimd.dma_gather(
                        out_ap=wloT[:],
                        in_ap=wlo_dram,
                        idxs_ap=idx_sb[:, r0 // 16 : (r0 + ncols) // 16],
                        num_idxs=ncols,
                        num_idxs_reg=nval_c,
                        elem_size=D_MODEL,
                        transpose=True,
                    )
                elif mode == "dma_nt":
                    whiT = whi_pool.tile([P, -(-ncols // P), D_MODEL], f16, tag="whiT")
                    nc.gpsimd.dma_gather(
                        out_ap=whiT[:],
                        in_ap=whi_dram,
                        idxs_ap=idx_sb[:, r0 // 16 : (r0 + ncols) // 16],
                        num_idxs=ncols,
                        num_idxs_reg=ncols,
                        elem_size=D_MODEL,
                        transpose=False,
                    )
                    wloT = wlo_pool.tile([P, -(-ncols // P), D_MODEL], f16, tag="wloT")
                    nc.gpsimd.dma_gather(
                        out_ap=wloT[:],
                        in_ap=wlo_dram,
                        idxs_ap=idx_sb[:, r0 // 16 : (r0 + ncols) // 16],
                        num_idxs=ncols,
                        num_idxs_reg=ncols,
                        elem_size=D_MODEL,
                        transpose=False,
                    )
                else:
                    whiT = whiT_c
                    wloT = wloT_c

                if mode in ("dma", "dma_nt"):
                    continue

                if mode in ("fused", "fused2"):
                    # One PSUM chain: rows 0-31 accumulate xh*whi (hi chain);
                    # rows 32-63 accumulate xl_s*whi (mm1) AND xh*wlo_s (mm2).
                    # The group is opened by mm1@k=0 (spans rows 0-63) and
                    # closed by mm1@k=31, so mm2@k=31 is emitted before it.
                    psAB = psum.tile([64, ncols], mybir.dt.float32, tag="psA")

                    def mm1(k):
                        nc.tensor.matmul(
                            out=psAB[:],
                            lhsT=xp_sb[:, k * 2 * B : (k + 1) * 2 * B],
                            rhs=whiT[:, k, :],
                            start=(k == 0),
                            stop=(k == KT - 1),
                        )

                    def mm2(k):
                        nc.tensor.matmul(
                            out=psAB[B : 2 * B, :],
                            lhsT=xp_sb[:, k * 2 * B : k * 2 * B + B],
                            rhs=wloT[:, k, :],
                            start=False,
                            stop=False,
                        )

                    # mm1s first: they only depend on the whi gather, so the
                    # PE starts before wlo lands. mm1@KT-1 closes the group.
                    for k in range(KT - 1):
                        mm1(k)
                    for k in range(KT):
                        mm2(k)
                    mm1(KT - 1)
                    if mode == "fused2":
                        # recombine on the PE: out = combM.T @ [hi; lo; 0]
                        # = hi + lo/2048, landing directly on partitions 0-31
                        cmb_sb = out_pool.tile([P, ncols], f32, tag="cmb")
                        if _rep == 0 and c < 2:
                            nc.vector.memset(cmb_sb[2 * B :, :], 0)
                        nc.vector.tensor_copy(cmb_sb[: 2 * B, :], psAB[:])
                        psO = psum.tile([B, ncols], mybir.dt.float32, tag="psO")
                        nc.tensor.matmul(
                            out=psO[:], lhsT=combM[:], rhs=cmb_sb[:],
                            start=True, stop=True,
                        )
                        nc.scalar.copy(out_sb[:, r0 : r0 + ncols], psO[:])
                        if not tiny_out:
                            nc.sync.dma_start(
                                out_dram[:, r0 : r0 + ncols],
                                out_sb[:, r0 : r0 + ncols],
                            )
                        continue
                    # hi chain copied to partitions 0-31; lo-sum scaled on
                    # 32-63, realigned to 0-31 with an HWDGE SBUF-SBUF DMA
                    # (keeps the tail off the SWDGE queue the gathers use),
                    # then added and written out per chunk.
                    nc.scalar.copy(outA_sb[:, r0 : r0 + ncols], psAB[:B, :])
                    nc.vector.tensor_scalar_mul(
                        t1_sb[B : 2 * B, r0 : r0 + ncols],
                        psAB[B : 2 * B, :],
                        1.0 / LO_SCALE,
                    )
                    nc.sync.dma_start(
                        out_sb[:, r0 : r0 + ncols],
                        t1_sb[B : 2 * B, r0 : r0 + ncols],
                    )
                    nc.vector.tensor_add(
                        out_sb[:, r0 : r0 + ncols],
                        out_sb[:, r0 : r0 + ncols],
                        outA_sb[:, r0 : r0 + ncols],
                    )
                    if not tiny_out:
                        nc.sync.dma_start(
                            out_dram[:, r0 : r0 + ncols],
                            out_sb[:, r0 : r0 + ncols],
                        )
                    continue

                # mode full/mm: 3 matmul passes, both chains on partitions 0-31
                mcols = 256 if mode == "mm" else ncols
                psA = psum.tile([B, mcols], mybir.dt.float32, tag="psA")
                psB = psum.tile([B, mcols], mybir.dt.float32, tag="psB")
                for k in range(KT):
                    xh_k = xh_sb[:, k * B : (k + 1) * B]
                    xl_k = xl_sb[:, k * B : (k + 1) * B]
                    nc.tensor.matmul(
                        out=psA[:],
                        lhsT=xh_k,
                        rhs=whiT[:, k, :mcols],
                        start=(k == 0),
                        stop=(k == KT - 1),
                    )
                    nc.tensor.matmul(
                        out=psB[:],
                        lhsT=xh_k,
                        rhs=wloT[:, k, :mcols],
                        start=(k == 0),
                        stop=False,
                    )
                    nc.tensor.matmul(
                        out=psB[:],
                        lhsT=xl_k,
                        rhs=whiT[:, k, :mcols],
                        start=False,
                        stop=(k == KT - 1),
                    )
                dst = out_sb[:, r0 : r0 + ncols]
                nc.scalar.mul(dst, psB[:, :ncols], 1.0 / LO_SCALE)
                nc.vector.tensor_add(dst, dst, psA[:, :ncols])

            if mode == "fused":
                if tiny_out:
                    nc.sync.dma_start(out_dram, outA_sb[:, :out_cols])
                continue
            if mode == "fused2":
                continue
            if mode in ("dma", "dma_nt"):
                nc.vector.tensor_copy(out_sb[:, :64], whiT[:32, 0, :64])
            nc.sync.dma_start(out_dram, out_sb[:, :out_cols])

    nc.compile()
    _cache[key] = nc
    return nc


def _prep_scatter(nc, out_dram, obs, idx_sb, nv16, c):
    B_ = B
    nc.gpsimd.dma_scatter_add(
        out_ap=out_dram,
        in_ap=obs[:, c : c + 1, :],
        idxs_ap=idx_sb[:, nv16 + c * 8 : nv16 + c * 8 + 8],
        num_idxs=128,
        num_idxs_reg=128,
        elem_size=2 * B_,
        prepare_only=True,
        queue_num=1,
    )


def _build_hi(nvalid, gbufs=3, reps=1):
    """fp16-only weight-stationary kernel.

    One fp16 gather per 128-row chunk lands matmul-ready [d%128, k, r].
    Matmuls are weight-stationary: lhsT = gathered chunk [128, ncols<=128],
    rhs = x^T fp16 [128, 32] per contraction tile, accumulating
    psT[r, b] over the 32 k-tiles. Output is written transposed
    [nvalid, B]; the host transposes during assembly. fp16 on both sides
    gives ~2.4e-4 scale-rel error (gate is 2e-2).
    """
    key = ("hi2", nvalid, gbufs, reps)
    if key in _cache:
        return _cache[key]
    from concourse import bacc, mybir, tile

    f32 = mybir.dt.float32
    f16 = mybir.dt.float16
    i16 = mybir.dt.int16

    nch = nvalid // 128
    nv16 = nvalid // 16

    nc = bacc.Bacc(
        "TRN2",
        target_bir_lowering=False,
        debug=False,
        enable_asserts=False,
        num_swdge_queues=2,
    )
    whi_dram = nc.dram_tensor("whi", [D_FF, D_MODEL], f16, kind="ExternalInput").ap()
    xh_dram = nc.dram_tensor("xh", [P, KT * B], f16, kind="ExternalInput").ap()
    # first nv16 cols: gather indices; next nv16: iota rows for the scatter
    idx_dram = nc.dram_tensor("idx", [P, 2 * nv16], i16, kind="ExternalInput").ap()
    # 64-wide rows so the scatter elem is 256B (cols 32..63 are zero pad)
    out_dram = nc.dram_tensor("out", [nvalid, 2 * B], f32, kind="ExternalOutput").ap()

    with tile.TileContext(nc) as tc, ExitStack() as ctx:
        consts = ctx.enter_context(tc.tile_pool(name="consts", bufs=1))
        whi_pool = ctx.enter_context(tc.tile_pool(name="whiT", bufs=max(gbufs, nch)))
        psum = ctx.enter_context(tc.tile_pool(name="psum", bufs=4, space="PSUM"))

        idx_sb = consts.tile([P, 2 * nv16], i16)
        nc.sync.dma_start(idx_sb[:], idx_dram)
        xh_sb = consts.tile([P, KT * B], f16)
        nc.sync.dma_start(xh_sb[:], xh_dram)
        obs = consts.tile([P, nch, 2 * B], f32)
        nc.vector.memset(obs[:], 0)

        # issue all gathers + output-scatter preps first so the Pool SEQ is
        # never blocked behind a trigger's data wait; desc-gen pipelines
        # ahead of the serialized DMA transfers
        whiTs = []
        for c in range(nch):
            r0 = c * 128
            whiT = whi_pool.tile([P, KT, 128], f16, tag=f"whiT{c}")
            nc.gpsimd.dma_gather(
                out_ap=whiT[:],
                in_ap=whi_dram,
                idxs_ap=idx_sb[:, r0 // 16 : r0 // 16 + 8],
                num_idxs=128,
                num_idxs_reg=128,
                elem_size=D_MODEL,
                transpose=True,
            )
            whiTs.append(whiT)

        for c in range(nch):
            whiT = whiTs[c]
            psT = psum.tile([128, B], f32, tag="psT")
            for k in range(KT):
                nc.tensor.matmul(
                    out=psT[:],
                    lhsT=whiT[:, k, :],
                    rhs=xh_sb[:, k * B : (k + 1) * B],
                    start=(k == 0),
                    stop=(k == KT - 1),
                )
            nc.scalar.copy(obs[:, c, :B], psT[:])
            nc.sync.dma_start(out_dram[c * 128 : (c + 1) * 128, :B], obs[:, c, :B])

    nc.compile()
    _cache[key] = nc
    return nc


def _make_in_maps_hi(x, weight, indices):
    """Host prep for the hi kernel: dedup+shard indices, fp16 casts.

    Returns (in_maps, assemble_fn, nvalid)."""
    x = np.asarray(x, dtype=np.float32)
    weight = np.asarray(weight, dtype=np.float32)
    indices = np.asarray(indices, dtype=np.int64)

    whi = np.ascontiguousarray(weight.astype(np.float16))
    xt = np.ascontiguousarray(
        x[:, 0, :].reshape(B, KT, P).transpose(2, 1, 0).reshape(P, KT * B)
    )
    xh = np.ascontiguousarray(xt.astype(np.float16))

    uniq, inv = np.unique(indices, return_inverse=True)
    nu = len(uniq)
    base, rem = divmod(nu, NCORES)
    counts = [base + (1 if c < rem else 0) for c in range(NCORES)]
    starts = np.concatenate([[0], np.cumsum(counts)[:-1]])
    # transpose dma_gather requires num_idxs % 128 == 0
    nvalid = -(-max(counts) // 128) * 128

    iota = _wrap_idx(np.arange(nvalid, dtype=np.int16))
    in_maps = []
    for c in range(NCORES):
        idx_pad = np.zeros(nvalid, dtype=np.int16)
        idx_pad[: counts[c]] = uniq[starts[c] : starts[c] + counts[c]]
        idx_full = np.concatenate([_wrap_idx(idx_pad), iota], axis=1)
        in_maps.append({"whi": whi, "xh": xh, "idx": np.ascontiguousarray(idx_full)})

    def assemble(results):
        cols = np.empty((B, nu), dtype=np.float32)
        for c in range(NCORES):
            cols[:, starts[c] : starts[c] + counts[c]] = (
                results[c]["out"][: counts[c], :B].T
            )
        return np.ascontiguousarray(cols[:, inv].reshape(B, 1, R_TOTAL))

    return in_maps, assemble, nvalid


def _split_pair(a):
    """fp32 -> (hi fp16, (a-hi)*2048 fp16). hi + lo/2048 ~= a to ~2^-22 rel."""
    hi = a.astype(np.float16)
    lo = ((a - hi.astype(np.float32)) * LO_SCALE).astype(np.float16)
    return hi, lo


def _wrap_idx(idx_pad):
    """[npad] int16 -> [128, npad//16] wrapped-16 layout, replicated 8x."""
    npad = idx_pad.shape[0]
    blk = idx_pad.reshape(npad // 16, 16).T  # [16, npad//16]
    return np.ascontiguousarray(np.tile(blk, (8, 1)))


def _make_in_maps(x, weight, indices, dedup=True):
    """Returns (in_maps, assemble_fn, npad)."""
    x = np.asarray(x, dtype=np.float32)
    weight = np.ascontiguousarray(np.asarray(weight, dtype=np.float32))
    indices = np.asarray(indices, dtype=np.int64)

    whi, wlo = _split_pair(weight)
    whi = np.ascontiguousarray(whi)
    wlo = np.ascontiguousarray(wlo)

    # x^T staged so the DMA is contiguous: xt[p, k*32+b] = x[b, 0, k*128+p]
    xt = np.ascontiguousarray(
        x[:, 0, :].reshape(B, KT, P).transpose(2, 1, 0).reshape(P, KT * B)
    )
    xh, xl = _split_pair(xt)
    # packed [xh || xl] per contraction tile for the fused M=64 matmul
    xp = np.empty((P, KT, 2 * B), dtype=np.float16)
    xp[:, :, :B] = xh.reshape(P, KT, B)
    xp[:, :, B:] = xl.reshape(P, KT, B)
    xp = np.ascontiguousarray(xp.reshape(P, KT * 2 * B))
    combm = np.zeros((P, B), dtype=np.float32)
    combm[:B, :] = np.eye(B, dtype=np.float32)
    combm[B : 2 * B, :] = np.eye(B, dtype=np.float32) / LO_SCALE

    uniq, inv = np.unique(indices, return_inverse=True)
    nu = len(uniq)
    use_dedup = dedup and -(-nu // NCORES) <= NPAD_DEDUP
    if use_dedup:
        npad = NPAD_DEDUP
        base, rem = divmod(nu, NCORES)
        counts = [base + (1 if c < rem else 0) for c in range(NCORES)]
        starts = np.concatenate([[0], np.cumsum(counts)[:-1]])
        core_idx = [uniq[starts[c] : starts[c] + counts[c]] for c in range(NCORES)]
    else:
        npad = NPAD
        counts = list(_CORE_N)
        starts = list(_CORE_START)
        core_idx = [
            indices[starts[c] : starts[c] + counts[c]] for c in range(NCORES)
        ]

    # uniform valid count (dup-padded with row 0); -1 beyond it is trimmed
    # from the gather transfer by the SWDGE
    nvalid = min(-(-max(counts) // 16) * 16, npad)

    in_maps = []
    for c in range(NCORES):
        idx_pad = np.full(npad, -1, dtype=np.int16)
        idx_pad[: counts[c]] = core_idx[c]
        idx_pad[counts[c] : nvalid] = 0
        in_maps.append(
            {
                "whi": whi,
                "wlo": wlo,
                "xh": np.ascontiguousarray(xh),
                "xl": np.ascontiguousarray(xl),
                "xp": xp,
                "combm": combm,
                "idx": _wrap_idx(idx_pad),
            }
        )

    def assemble(results):
        cols = np.empty((B, sum(counts)), dtype=np.float32)
        for c in range(NCORES):
            cols[:, starts[c] : starts[c] + counts[c]] = results[c]["out"][
                :, : counts[c]
            ]
        if use_dedup:
            out = cols[:, inv]
        else:
            out = cols
        return np.ascontiguousarray(out.reshape(B, 1, R_TOTAL))

    return in_maps, assemble, npad, nvalid


def _filter_in_maps(nc, in_maps):
    names = set()
    from concourse import mybir

    for alloc in nc.m.functions[0].allocations:
        if isinstance(alloc, mybir.MemoryLocationSet) and alloc.kind == "ExternalInput":
            names.add(alloc.memorylocations[0].name)
    return [{k: v for k, v in m.items() if k in names} for m in in_maps]


def run_full(x, weight, indices, trace=False, mode="hi", dedup=True):
    """Returns (output, BassKernelResults)."""
    from concourse.bass_utils import run_bass_kernel_spmd

    if mode == "hi":
        in_maps, assemble, nvalid = _make_in_maps_hi(x, weight, indices)
        nc = _build_hi(nvalid)
        in_maps = _filter_in_maps(nc, in_maps)
        res = run_bass_kernel_spmd(nc, in_maps, list(range(NCORES)), trace=trace)
        return assemble(res.results), res

    in_maps, assemble, npad, nvalid = _make_in_maps(x, weight, indices, dedup=dedup)
    if mode == "fused2" and npad != NPAD_DEDUP:
        # the fused2 epilogue is only validated for the 512-wide dedup
        # layout; the rare >4096-unique fallback uses the fused tail
        mode = "fused"
    nc = _build(1, mode, False, npad, nvalid=nvalid)
    in_maps = _filter_in_maps(nc, in_maps)
    res = run_bass_kernel_spmd(nc, in_maps, list(range(NCORES)), trace=trace)
    return assemble(res.results), res


def kernel(x, weight, indices):
    out, _ = run_full(x, weight, indices)
    return out



# revision 18
# speedup vs baseline: 1.8355x; 1.0468x over previous
"""Trainium2 Bass kernel for gathered-row MLP decode matmul.

out[b, 0, r] = sum_d x[b, 0, d] * weight[indices[r], d]

Strategy: dedup+sort the indices on the host, shard them contiguously
across 8 cores, pad per-core to a fixed multiple of 128. The fp32 weight is
split on the host into an fp16 hi/lo pair (residual pre-scaled by 2^11 to
stay in fp16 normal range; hi + lo/2048 reconstructs fp32 to ~2^-22).
Each core gathers its rows of both halves with dma_gather(transpose=True),
which lands them directly in matmul-ready [d%128, d//128, r] layout — no
on-chip transposes. The x operand is pre-transposed and hi/lo-split on the
host and packed [xh||xl] so one M=64 matmul computes xh*whi and xl*whi in a
single moving pass; a second M=32 matmul adds xh*wlo. The two lo-products
are summed on partitions 32-63, realigned to 0-31 with a small SBUF-SBUF
DMA, scaled by 2^-11 and added to the hi chain — fp32-class accuracy with
2 moving passes per contraction tile. Host scatters the per-core unique
outputs back to the original 4403 index order.
"""

import os
import sys
from contextlib import ExitStack

sys.path.insert(0, "/opt/trn_rl_repo")
os.environ.setdefault("MYCRO_LOCAL_CACHE", "1")

import numpy as np

D_FF = 11008
D_MODEL = 4096
R_TOTAL = 4403
B = 32
NCORES = 8
P = 128
KT = D_MODEL // P          # 32 contraction tiles
NPAD = 640                 # padded per-core index count (5*128), fallback
NPAD_DEDUP = 512           # padded per-core count for the dedup path
LO_SCALE = 2048.0          # wlo/xlo pre-scale (2^11)

# per-core share of the real 4403 indices (no-dedup fallback)
_CORE_N = [551, 551, 551, 550, 550, 550, 550, 550]
_CORE_START = [0]
for _n in _CORE_N[:-1]:
    _CORE_START.append(_CORE_START[-1] + _n)

_cache = {}


def _build(
    reps=1, mode="full", tiny_out=False, npad=NPAD, chunks=None, gbufs=2,
    nvalid=None,
):
    """mode: full (3-matmul) | fused (2-pass M=64 packing) | dma (gathers
    only) | dma_nt (non-transpose gathers) | mm (matmuls only).
    tiny_out: shrink the DRAM output to [B, 64] so bench-loop host
    transfers are negligible (timing only)."""
    if nvalid is None:
        nvalid = npad
    key = ("nc", reps, mode, tiny_out, npad, chunks, gbufs, nvalid)
    if key in _cache:
        return _cache[key]
    from concourse import bacc, mybir, tile

    f32 = mybir.dt.float32
    f16 = mybir.dt.float16
    i16 = mybir.dt.int16

    if chunks is None:
        chunks = tuple((i, min(256, npad - i)) for i in range(0, npad, 256))

    nc = bacc.Bacc(
        "TRN2", target_bir_lowering=False, debug=False, enable_asserts=False
    )
    whi_dram = nc.dram_tensor("whi", [D_FF, D_MODEL], f16, kind="ExternalInput").ap()
    wlo_dram = nc.dram_tensor("wlo", [D_FF, D_MODEL], f16, kind="ExternalInput").ap()
    if mode in ("fused", "fused2"):
        xp_dram = nc.dram_tensor("xp", [P, KT * 2 * B], f16, kind="ExternalInput").ap()
        if mode == "fused2":
            combm_dram = nc.dram_tensor(
                "combm", [P, B], f32, kind="ExternalInput"
            ).ap()
    else:
        xh_dram = nc.dram_tensor("xh", [P, KT * B], f16, kind="ExternalInput").ap()
        xl_dram = nc.dram_tensor("xl", [P, KT * B], f16, kind="ExternalInput").ap()
    idx_dram = nc.dram_tensor("idx", [P, npad // 16], i16, kind="ExternalInput").ap()
    out_cols = 64 if tiny_out else npad
    out_dram = nc.dram_tensor("out", [B, out_cols], f32, kind="ExternalOutput").ap()

    with tile.TileContext(nc) as tc, ExitStack() as ctx:
        consts = ctx.enter_context(tc.tile_pool(name="consts", bufs=1))
        whi_pool = ctx.enter_context(tc.tile_pool(name="whiT", bufs=gbufs))
        wlo_pool = ctx.enter_context(tc.tile_pool(name="wloT", bufs=gbufs))
        psum = ctx.enter_context(tc.tile_pool(name="psum", bufs=4, space="PSUM"))
        out_pool = ctx.enter_context(tc.tile_pool(name="outp", bufs=2))

        # idx first: the gathers (the critical path) depend only on it
        idx_sb = consts.tile([P, npad // 16], i16)
        nc.sync.dma_start(idx_sb[:], idx_dram)
        if mode in ("fused", "fused2"):
            xp_sb = consts.tile([P, KT * 2 * B], f16)
            nc.sync.dma_start(xp_sb[:], xp_dram)
        else:
            xh_sb = consts.tile([P, KT * B], f16)
            nc.sync.dma_start(xh_sb[:], xh_dram)
            xl_sb = consts.tile([P, KT * B], f16)
            nc.sync.dma_start(xl_sb[:], xl_dram)

        if mode == "mm":
            whiT_c = consts.tile([P, KT, 256], f16)
            nc.gpsimd.memset(whiT_c[:], 0.25)
            wloT_c = consts.tile([P, KT, 256], f16)
            nc.gpsimd.memset(wloT_c[:], 0.25)

        if mode == "fused2":
            # combM.T @ [hi; lo; 0] = hi + lo/2048 (host-built constant;
            # K padded to 128 so the fp32 matmul uses the proven full
            # partition-group shape)
            combM = consts.tile([P, B], f32)
            nc.sync.dma_start(combM[:], combm_dram)

        for _rep in range(reps):
            out_sb = out_pool.tile([B, npad], f32, tag="out_sb")
            if mode == "fused":
                t1_sb = out_pool.tile([64, npad], f32, tag="t1")
                outA_sb = out_pool.tile([B, npad], f32, tag="outA")

            for c, (r0, ncols) in enumerate(chunks):
                if mode in ("full", "fused", "fused2", "dma"):
                    # valid (non -1) indices in this chunk; the SWDGE trims
                    # the transfer to the valid prefix, so -1 tail padding
                    # costs no DMA.
                    nval_c = max(0, min(nvalid - r0, ncols))
                    # whiT[p, k, i] = whi[idx[r0+i], k*128 + p]
                    whiT = whi_pool.tile([P, KT, ncols], f16, tag="whiT")
                    wloT = wlo_pool.tile([P, KT, ncols], f16, tag="wloT")
                    if _rep == 0 and nval_c < ncols:
                        # first use of the slot: zero the never-gathered tail
                        # columns so downstream matmuls read finite data
                        nc.vector.memset(whiT[:, :, nval_c:], 0)
                        nc.vector.memset(wloT[:, :, nval_c:], 0)
                    nc.gpsimd.dma_gather(
                        out_ap=whiT[:],
                        in_ap=whi_dram,
                        idxs_ap=idx_sb[:, r0 // 16 : (r0 + ncols) // 16],
                        num_idxs=ncols,
                        num_idxs_reg=nval_c,
                        elem_size=D_MODEL,
                        transpose=True,
                    )
                    nc.gpsimd.dma_gather(
                        out_ap=wloT[:],
                        in_ap=wlo_dram,
                        idxs_ap=idx_sb[:, r0 // 16 : (r0 + ncols) // 16],
                        num_idxs=ncols,
                        num_idxs_reg=nval_c,
                        elem_size=D_MODEL,
                        transpose=True,
                    )
                elif mode == "dma_nt":
                    whiT = whi_pool.tile([P, -(-ncols // P), D_MODEL], f16, tag="whiT")
                    nc.gpsimd.dma_gather(
                        out_ap=whiT[:],
                        in_ap=whi_dram,
                        idxs_ap=idx_sb[:, r0 // 16 : (r0 + ncols) // 16],
                        num_idxs=ncols,
                        num_idxs_reg=ncols,
                        elem_size=D_MODEL,
                        transpose=False,
                    )
                    wloT = wlo_pool.tile([P, -(-ncols // P), D_MODEL], f16, tag="wloT")
                    nc.gpsimd.dma_gather(
                        out_ap=wloT[:],
                        in_ap=wlo_dram,
                        idxs_ap=idx_sb[:, r0 // 16 : (r0 + ncols) // 16],
                        num_idxs=ncols,
                        num_idxs_reg=ncols,
                        elem_size=D_MODEL,
                        transpose=False,
                    )
                else:
                    whiT = whiT_c
                    wloT = wloT_c

                if mode in ("dma", "dma_nt"):
                    continue

                if mode in ("fused", "fused2"):
                    # One PSUM chain: rows 0-31 accumulate xh*whi (hi chain);
                    # rows 32-63 accumulate xl_s*whi (mm1) AND xh*wlo_s (mm2).
                    # The group is opened by mm1@k=0 (spans rows 0-63) and
                    # closed by mm1@k=31, so mm2@k=31 is emitted before it.
                    psAB = psum.tile([64, ncols], mybir.dt.float32, tag="psA")

                    def mm1(k):
                        nc.tensor.matmul(
                            out=psAB[:],
                            lhsT=xp_sb[:, k * 2 * B : (k + 1) * 2 * B],
                            rhs=whiT[:, k, :],
                            start=(k == 0),
                            stop=(k == KT - 1),
                        )

                    def mm2(k):
                        nc.tensor.matmul(
                            out=psAB[B : 2 * B, :],
                            lhsT=xp_sb[:, k * 2 * B : k * 2 * B + B],
                            rhs=wloT[:, k, :],
                            start=False,
                            stop=False,
                        )

                    # mm1s first: they only depend on the whi gather, so the
                    # PE starts before wlo lands. mm1@KT-1 closes the group.
                    for k in range(KT - 1):
                        mm1(k)
                    for k in range(KT):
                        mm2(k)
                    mm1(KT - 1)
                    if mode == "fused2":
                        # recombine on the PE: out = combM.T @ [hi; lo; 0]
                        # = hi + lo/2048, landing directly on partitions 0-31
                        cmb_sb = out_pool.tile([P, ncols], f32, tag="cmb")
                        if _rep == 0 and c < 2:
                            nc.vector.memset(cmb_sb[2 * B :, :], 0)
                        nc.vector.tensor_copy(cmb_sb[: 2 * B, :], psAB[:])
                        psO = psum.tile([B, ncols], mybir.dt.float32, tag="psO")
                        nc.tensor.matmul(
                            out=psO[:], lhsT=combM[:], rhs=cmb_sb[:],
                            start=True, stop=True,
                        )
                        nc.scalar.copy(out_sb[:, r0 : r0 + ncols], psO[:])
                        if not tiny_out:
                            nc.sync.dma_start(
                                out_dram[:, r0 : r0 + ncols],
                                out_sb[:, r0 : r0 + ncols],
                            )
                        continue
                    # hi chain copied to partitions 0-31; lo-sum scaled on
                    # 32-63, realigned to 0-31 with an HWDGE SBUF-SBUF DMA
                    # (keeps the tail off the SWDGE queue the gathers use),
                    # then added and written out per chunk.
                    nc.scalar.copy(outA_sb[:, r0 : r0 + ncols], psAB[:B, :])
                    nc.vector.tensor_scalar_mul(
                        t1_sb[B : 2 * B, r0 : r0 + ncols],
                        psAB[B : 2 * B, :],
                        1.0 / LO_SCALE,
                    )
                    nc.sync.dma_start(
                        out_sb[:, r0 : r0 + ncols],
                        t1_sb[B : 2 * B, r0 : r0 + ncols],
                    )
                    nc.vector.tensor_add(
                        out_sb[:, r0 : r0 + ncols],
                        out_sb[:, r0 : r0 + ncols],
                        outA_sb[:, r0 : r0 + ncols],
                    )
                    if not tiny_out:
                        nc.sync.dma_start(
                            out_dram[:, r0 : r0 + ncols],
                            out_sb[:, r0 : r0 + ncols],
                        )
                    continue

                # mode full/mm: 3 matmul passes, both chains on partitions 0-31
                mcols = 256 if mode == "mm" else ncols
                psA = psum.tile([B, mcols], mybir.dt.float32, tag="psA")
                psB = psum.tile([B, mcols], mybir.dt.float32, tag="psB")
                for k in range(KT):
                    xh_k = xh_sb[:, k * B : (k + 1) * B]
                    xl_k = xl_sb[:, k * B : (k + 1) * B]
                    nc.tensor.matmul(
                        out=psA[:],
                        lhsT=xh_k,
                        rhs=whiT[:, k, :mcols],
                        start=(k == 0),
                        stop=(k == KT - 1),
                    )
                    nc.tensor.matmul(
                        out=psB[:],
                        lhsT=xh_k,
                        rhs=wloT[:, k, :mcols],
                        start=(k == 0),
                        stop=False,
                    )
                    nc.tensor.matmul(
                        out=psB[:],
                        lhsT=xl_k,
                        rhs=whiT[:, k, :mcols],
                        start=False,
                        stop=(k == KT - 1),
                    )
                dst = out_sb[:, r0 : r0 + ncols]
                nc.scalar.mul(dst, psB[:, :ncols], 1.0 / LO_SCALE)
                nc.vector.tensor_add(dst, dst, psA[:, :ncols])

            if mode == "fused":
                if tiny_out:
                    nc.sync.dma_start(out_dram, outA_sb[:, :out_cols])
                continue
            if mode == "fused2":
                continue
            if mode in ("dma", "dma_nt"):
                nc.vector.tensor_copy(out_sb[:, :64], whiT[:32, 0, :64])
            nc.sync.dma_start(out_dram, out_sb[:, :out_cols])

    nc.compile()
    _cache[key] = nc
    return nc


def _prep_scatter(nc, out_dram, obs, idx_sb, nv16, c):
    B_ = B
    nc.gpsimd.dma_scatter_add(
        out_ap=out_dram,
        in_ap=obs[:, c : c + 1, :],
        idxs_ap=idx_sb[:, nv16 + c * 8 : nv16 + c * 8 + 8],
        num_idxs=128,
        num_idxs_reg=128,
        elem_size=2 * B_,
        prepare_only=True,
        queue_num=1,
    )


def _build_hi(nvalid, gbufs=3, reps=1):
    """fp16-only weight-stationary kernel.

    One fp16 gather per 128-row chunk lands matmul-ready [d%128, k, r].
    Matmuls are weight-stationary: lhsT = gathered chunk [128, ncols<=128],
    rhs = x^T fp16 [128, 32] per contraction tile, accumulating
    psT[r, b] over the 32 k-tiles. Output is written transposed
    [nvalid, B]; the host transposes during assembly. fp16 on both sides
    gives ~2.4e-4 scale-rel error (gate is 2e-2).
    """
    key = ("hi3", nvalid, gbufs, reps)
    if key in _cache:
        return _cache[key]
    from concourse import bacc, mybir, tile

    f32 = mybir.dt.float32
    f16 = mybir.dt.float16
    i16 = mybir.dt.int16

    nfull = nvalid // 128      # full 128-row transpose-gather chunks
    nt = nvalid % 128          # odd-size tail chunk, non-transpose gather
    nch = nfull + (1 if nt else 0)
    nv16 = nvalid // 16

    nc = bacc.Bacc(
        "TRN2",
        target_bir_lowering=False,
        debug=False,
        enable_asserts=False,
        num_swdge_queues=2,
    )
    whi_dram = nc.dram_tensor("whi", [D_FF, D_MODEL], f16, kind="ExternalInput").ap()
    xh_dram = nc.dram_tensor("xh", [P, KT * B], f16, kind="ExternalInput").ap()
    if nt:
        id_dram = nc.dram_tensor("ident", [nt, nt], f16, kind="ExternalInput").ap()
    # first nv16 cols: gather indices; next nv16: iota rows for the scatter
    idx_dram = nc.dram_tensor("idx", [P, 2 * nv16], i16, kind="ExternalInput").ap()
    # 64-wide rows so the scatter elem is 256B (cols 32..63 are zero pad)
    out_dram = nc.dram_tensor("out", [nvalid, 2 * B], f32, kind="ExternalOutput").ap()

    with tile.TileContext(nc) as tc, ExitStack() as ctx:
        consts = ctx.enter_context(tc.tile_pool(name="consts", bufs=1))
        whi_pool = ctx.enter_context(tc.tile_pool(name="whiT", bufs=max(gbufs, nch)))
        psum = ctx.enter_context(tc.tile_pool(name="psum", bufs=4, space="PSUM"))

        idx_sb = consts.tile([P, 2 * nv16], i16)
        nc.sync.dma_start(idx_sb[:], idx_dram)
        xh_sb = consts.tile([P, KT * B], f16)
        nc.sync.dma_start(xh_sb[:], xh_dram)
        obs = consts.tile([P, nch, 2 * B], f32)
        nc.vector.memset(obs[:], 0)
        if nt:
            id_sb = consts.tile([nt, nt], f16)
            nc.sync.dma_start(id_sb[:], id_dram)
            w4 = consts.tile([P, 1, D_MODEL], f16)
            nc.vector.memset(w4[:], 0)

        # issue all gathers + output-scatter preps first so the Pool SEQ is
        # never blocked behind a trigger's data wait; desc-gen pipelines
        # ahead of the serialized DMA transfers
        whiTs = []
        if nt:
            # odd tail gathered FIRST (its transfer is the shortest and its
            # PE transposes hide under the remaining gathers' transfers)
            nc.gpsimd.dma_gather(
                out_ap=w4[:],
                in_ap=whi_dram,
                idxs_ap=idx_sb[:, nfull * 8 : nv16],
                num_idxs=nt,
                num_idxs_reg=nt,
                elem_size=D_MODEL,
                transpose=False,
            )
        for c in range(nfull):
            r0 = c * 128
            whiT = whi_pool.tile([P, KT, 128], f16, tag=f"whiT{c}")
            nc.gpsimd.dma_gather(
                out_ap=whiT[:],
                in_ap=whi_dram,
                idxs_ap=idx_sb[:, r0 // 16 : r0 // 16 + 8],
                num_idxs=128,
                num_idxs_reg=128,
                elem_size=D_MODEL,
                transpose=True,
            )
            whiTs.append(whiT)
        if nt:
            # on-chip transpose of the tail rows into matmul-ready layout
            whiT_nt = whi_pool.tile([P, KT, nt], f16, tag="whiTnt")
            for k in range(KT):
                psX = psum.tile([P, nt], f16, tag="psX")
                nc.tensor.transpose(
                    psX[:], w4[:nt, 0, k * P : (k + 1) * P], id_sb[:]
                )
                nc.vector.tensor_copy(whiT_nt[:, k, :], psX[:])
            whiTs.append(whiT_nt)

        order = list(range(nch))
        if nt and nch >= 2:
            # whiTs[nch-1] is the nt chunk (data ready early); run it before
            # the last full chunk, whose gather finishes last
            order = order[: nch - 2] + [nch - 1, nch - 2]
        for c in order:
            whiT = whiTs[c]
            rows = nt if (nt and c == nch - 1) else 128
            r0 = nfull * 128 if (nt and c == nch - 1) else c * 128
            psT = psum.tile([rows, B], f32, tag="psT")
            for k in range(KT):
                nc.tensor.matmul(
                    out=psT[:],
                    lhsT=whiT[:, k, :],
                    rhs=xh_sb[:, k * B : (k + 1) * B],
                    start=(k == 0),
                    stop=(k == KT - 1),
                )
            nc.scalar.copy(obs[:rows, c, :B], psT[:])
            nc.sync.dma_start(out_dram[r0 : r0 + rows, :B], obs[:rows, c, :B])

    nc.compile()
    _cache[key] = nc
    return nc


def _make_in_maps_hi(x, weight, indices):
    """Host prep for the hi kernel: dedup+shard indices, fp16 casts.

    Returns (in_maps, assemble_fn, nvalid)."""
    x = np.asarray(x, dtype=np.float32)
    weight = np.asarray(weight, dtype=np.float32)
    indices = np.asarray(indices, dtype=np.int64)

    whi = np.ascontiguousarray(weight.astype(np.float16))
    xt = np.ascontiguousarray(
        x[:, 0, :].reshape(B, KT, P).transpose(2, 1, 0).reshape(P, KT * B)
    )
    xh = np.ascontiguousarray(xt.astype(np.float16))

    uniq, inv = np.unique(indices, return_inverse=True)
    nu = len(uniq)
    base, rem = divmod(nu, NCORES)
    counts = [base + (1 if c < rem else 0) for c in range(NCORES)]
    starts = np.concatenate([[0], np.cumsum(counts)[:-1]])
    # multiples of 128 go through transpose gathers; the %16 tail is a
    # non-transpose gather + on-chip PE transpose
    nvalid = -(-max(counts) // 16) * 16

    iota = _wrap_idx(np.arange(nvalid, dtype=np.int16))
    ident = np.eye(nvalid % 128 or 1, dtype=np.float16)
    in_maps = []
    for c in range(NCORES):
        idx_pad = np.zeros(nvalid, dtype=np.int16)
        idx_pad[: counts[c]] = uniq[starts[c] : starts[c] + counts[c]]
        idx_full = np.concatenate([_wrap_idx(idx_pad), iota], axis=1)
        in_maps.append(
            {
                "whi": whi,
                "xh": xh,
                "idx": np.ascontiguousarray(idx_full),
                "ident": ident,
            }
        )

    def assemble(results):
        cols = np.empty((B, nu), dtype=np.float32)
        for c in range(NCORES):
            cols[:, starts[c] : starts[c] + counts[c]] = (
                results[c]["out"][: counts[c], :B].T
            )
        return np.ascontiguousarray(cols[:, inv].reshape(B, 1, R_TOTAL))

    return in_maps, assemble, nvalid


def _split_pair(a):
    """fp32 -> (hi fp16, (a-hi)*2048 fp16). hi + lo/2048 ~= a to ~2^-22 rel."""
    hi = a.astype(np.float16)
    lo = ((a - hi.astype(np.float32)) * LO_SCALE).astype(np.float16)
    return hi, lo


def _wrap_idx(idx_pad):
    """[npad] int16 -> [128, npad//16] wrapped-16 layout, replicated 8x."""
    npad = idx_pad.shape[0]
    blk = idx_pad.reshape(npad // 16, 16).T  # [16, npad//16]
    return np.ascontiguousarray(np.tile(blk, (8, 1)))


def _make_in_maps(x, weight, indices, dedup=True):
    """Returns (in_maps, assemble_fn, npad)."""
    x = np.asarray(x, dtype=np.float32)
    weight = np.ascontiguousarray(np.asarray(weight, dtype=np.float32))
    indices = np.asarray(indices, dtype=np.int64)

    whi, wlo = _split_pair(weight)
    whi = np.ascontiguousarray(whi)
    wlo = np.ascontiguousarray(wlo)

    # x^T staged so the DMA is contiguous: xt[p, k*32+b] = x[b, 0, k*128+p]
    xt = np.ascontiguousarray(
        x[:, 0, :].reshape(B, KT, P).transpose(2, 1, 0).reshape(P, KT * B)
    )
    xh, xl = _split_pair(xt)
    # packed [xh || xl] per contraction tile for the fused M=64 matmul
    xp = np.empty((P, KT, 2 * B), dtype=np.float16)
    xp[:, :, :B] = xh.reshape(P, KT, B)
    xp[:, :, B:] = xl.reshape(P, KT, B)
    xp = np.ascontiguousarray(xp.reshape(P, KT * 2 * B))
    combm = np.zeros((P, B), dtype=np.float32)
    combm[:B, :] = np.eye(B, dtype=np.float32)
    combm[B : 2 * B, :] = np.eye(B, dtype=np.float32) / LO_SCALE

    uniq, inv = np.unique(indices, return_inverse=True)
    nu = len(uniq)
    use_dedup = dedup and -(-nu // NCORES) <= NPAD_DEDUP
    if use_dedup:
        npad = NPAD_DEDUP
        base, rem = divmod(nu, NCORES)
        counts = [base + (1 if c < rem else 0) for c in range(NCORES)]
        starts = np.concatenate([[0], np.cumsum(counts)[:-1]])
        core_idx = [uniq[starts[c] : starts[c] + counts[c]] for c in range(NCORES)]
    else:
        npad = NPAD
        counts = list(_CORE_N)
        starts = list(_CORE_START)
        core_idx = [
            indices[starts[c] : starts[c] + counts[c]] for c in range(NCORES)
        ]

    # uniform valid count (dup-padded with row 0); -1 beyond it is trimmed
    # from the gather transfer by the SWDGE
    nvalid = min(-(-max(counts) // 16) * 16, npad)

    in_maps = []
    for c in range(NCORES):
        idx_pad = np.full(npad, -1, dtype=np.int16)
        idx_pad[: counts[c]] = core_idx[c]
        idx_pad[counts[c] : nvalid] = 0
        in_maps.append(
            {
                "whi": whi,
                "wlo": wlo,
                "xh": np.ascontiguousarray(xh),
                "xl": np.ascontiguousarray(xl),
                "xp": xp,
                "combm": combm,
                "idx": _wrap_idx(idx_pad),
            }
        )

    def assemble(results):
        cols = np.empty((B, sum(counts)), dtype=np.float32)
        for c in range(NCORES):
            cols[:, starts[c] : starts[c] + counts[c]] = results[c]["out"][
                :, : counts[c]
            ]
        if use_dedup:
            out = cols[:, inv]
        else:
            out = cols
        return np.ascontiguousarray(out.reshape(B, 1, R_TOTAL))

    return in_maps, assemble, npad, nvalid


def _filter_in_maps(nc, in_maps):
    names = set()
    from concourse import mybir

    for alloc in nc.m.functions[0].allocations:
        if isinstance(alloc, mybir.MemoryLocationSet) and alloc.kind == "ExternalInput":
            names.add(alloc.memorylocations[0].name)
    return [{k: v for k, v in m.items() if k in names} for m in in_maps]


def run_full(x, weight, indices, trace=False, mode="hi", dedup=True):
    """Returns (output, BassKernelResults)."""
    from concourse.bass_utils import run_bass_kernel_spmd

    if mode == "hi":
        in_maps, assemble, nvalid = _make_in_maps_hi(x, weight, indices)
        nc = _build_hi(nvalid)
        in_maps = _filter_in_maps(nc, in_maps)
        res = run_bass_kernel_spmd(nc, in_maps, list(range(NCORES)), trace=trace)
        return assemble(res.results), res

    in_maps, assemble, npad, nvalid = _make_in_maps(x, weight, indices, dedup=dedup)
    if mode == "fused2" and npad != NPAD_DEDUP:
        # the fused2 epilogue is only validated for the 512-wide dedup
        # layout; the rare >4096-unique fallback uses the fused tail
        mode = "fused"
    nc = _build(1, mode, False, npad, nvalid=nvalid)
    in_maps = _filter_in_maps(nc, in_maps)
    res = run_bass_kernel_spmd(nc, in_maps, list(range(NCORES)), trace=trace)
    return assemble(res.results), res


def kernel(x, weight, indices):
    out, _ = run_full(x, weight, indices)
    return out



# revision 19
# speedup vs baseline: 1.8516x; 1.0088x over previous
"""Trainium2 Bass kernel for gathered-row MLP decode matmul.

out[b, 0, r] = sum_d x[b, 0, d] * weight[indices[r], d]

Strategy: dedup+sort the indices on the host, shard them contiguously
across 8 cores, pad per-core to a fixed multiple of 128. The fp32 weight is
split on the host into an fp16 hi/lo pair (residual pre-scaled by 2^11 to
stay in fp16 normal range; hi + lo/2048 reconstructs fp32 to ~2^-22).
Each core gathers its rows of both halves with dma_gather(transpose=True),
which lands them directly in matmul-ready [d%128, d//128, r] layout — no
on-chip transposes. The x operand is pre-transposed and hi/lo-split on the
host and packed [xh||xl] so one M=64 matmul computes xh*whi and xl*whi in a
single moving pass; a second M=32 matmul adds xh*wlo. The two lo-products
are summed on partitions 32-63, realigned to 0-31 with a small SBUF-SBUF
DMA, scaled by 2^-11 and added to the hi chain — fp32-class accuracy with
2 moving passes per contraction tile. Host scatters the per-core unique
outputs back to the original 4403 index order.
"""

import os
import sys
from contextlib import ExitStack

sys.path.insert(0, "/opt/trn_rl_repo")
os.environ.setdefault("MYCRO_LOCAL_CACHE", "1")

import numpy as np

D_FF = 11008
D_MODEL = 4096
R_TOTAL = 4403
B = 32
NCORES = 8
P = 128
KT = D_MODEL // P          # 32 contraction tiles
NPAD = 640                 # padded per-core index count (5*128), fallback
NPAD_DEDUP = 512           # padded per-core count for the dedup path
LO_SCALE = 2048.0          # wlo/xlo pre-scale (2^11)

# per-core share of the real 4403 indices (no-dedup fallback)
_CORE_N = [551, 551, 551, 550, 550, 550, 550, 550]
_CORE_START = [0]
for _n in _CORE_N[:-1]:
    _CORE_START.append(_CORE_START[-1] + _n)

_cache = {}


def _build(
    reps=1, mode="full", tiny_out=False, npad=NPAD, chunks=None, gbufs=2,
    nvalid=None,
):
    """mode: full (3-matmul) | fused (2-pass M=64 packing) | dma (gathers
    only) | dma_nt (non-transpose gathers) | mm (matmuls only).
    tiny_out: shrink the DRAM output to [B, 64] so bench-loop host
    transfers are negligible (timing only)."""
    if nvalid is None:
        nvalid = npad
    key = ("nc", reps, mode, tiny_out, npad, chunks, gbufs, nvalid)
    if key in _cache:
        return _cache[key]
    from concourse import bacc, mybir, tile

    f32 = mybir.dt.float32
    f16 = mybir.dt.float16
    i16 = mybir.dt.int16

    if chunks is None:
        chunks = tuple((i, min(256, npad - i)) for i in range(0, npad, 256))

    nc = bacc.Bacc(
        "TRN2", target_bir_lowering=False, debug=False, enable_asserts=False
    )
    whi_dram = nc.dram_tensor("whi", [D_FF, D_MODEL], f16, kind="ExternalInput").ap()
    wlo_dram = nc.dram_tensor("wlo", [D_FF, D_MODEL], f16, kind="ExternalInput").ap()
    if mode in ("fused", "fused2"):
        xp_dram = nc.dram_tensor("xp", [P, KT * 2 * B], f16, kind="ExternalInput").ap()
        if mode == "fused2":
            combm_dram = nc.dram_tensor(
                "combm", [P, B], f32, kind="ExternalInput"
            ).ap()
    else:
        xh_dram = nc.dram_tensor("xh", [P, KT * B], f16, kind="ExternalInput").ap()
        xl_dram = nc.dram_tensor("xl", [P, KT * B], f16, kind="ExternalInput").ap()
    idx_dram = nc.dram_tensor("idx", [P, npad // 16], i16, kind="ExternalInput").ap()
    out_cols = 64 if tiny_out else npad
    out_dram = nc.dram_tensor("out", [B, out_cols], f32, kind="ExternalOutput").ap()

    with tile.TileContext(nc) as tc, ExitStack() as ctx:
        consts = ctx.enter_context(tc.tile_pool(name="consts", bufs=1))
        whi_pool = ctx.enter_context(tc.tile_pool(name="whiT", bufs=gbufs))
        wlo_pool = ctx.enter_context(tc.tile_pool(name="wloT", bufs=gbufs))
        psum = ctx.enter_context(tc.tile_pool(name="psum", bufs=4, space="PSUM"))
        out_pool = ctx.enter_context(tc.tile_pool(name="outp", bufs=2))

        # idx first: the gathers (the critical path) depend only on it
        idx_sb = consts.tile([P, npad // 16], i16)
        nc.sync.dma_start(idx_sb[:], idx_dram)
        if mode in ("fused", "fused2"):
            xp_sb = consts.tile([P, KT * 2 * B], f16)
            nc.sync.dma_start(xp_sb[:], xp_dram)
        else:
            xh_sb = consts.tile([P, KT * B], f16)
            nc.sync.dma_start(xh_sb[:], xh_dram)
            xl_sb = consts.tile([P, KT * B], f16)
            nc.sync.dma_start(xl_sb[:], xl_dram)

        if mode == "mm":
            whiT_c = consts.tile([P, KT, 256], f16)
            nc.gpsimd.memset(whiT_c[:], 0.25)
            wloT_c = consts.tile([P, KT, 256], f16)
            nc.gpsimd.memset(wloT_c[:], 0.25)

        if mode == "fused2":
            # combM.T @ [hi; lo; 0] = hi + lo/2048 (host-built constant;
            # K padded to 128 so the fp32 matmul uses the proven full
            # partition-group shape)
            combM = consts.tile([P, B], f32)
            nc.sync.dma_start(combM[:], combm_dram)

        for _rep in range(reps):
            out_sb = out_pool.tile([B, npad], f32, tag="out_sb")
            if mode == "fused":
                t1_sb = out_pool.tile([64, npad], f32, tag="t1")
                outA_sb = out_pool.tile([B, npad], f32, tag="outA")

            for c, (r0, ncols) in enumerate(chunks):
                if mode in ("full", "fused", "fused2", "dma"):
                    # valid (non -1) indices in this chunk; the SWDGE trims
                    # the transfer to the valid prefix, so -1 tail padding
                    # costs no DMA.
                    nval_c = max(0, min(nvalid - r0, ncols))
                    # whiT[p, k, i] = whi[idx[r0+i], k*128 + p]
                    whiT = whi_pool.tile([P, KT, ncols], f16, tag="whiT")
                    wloT = wlo_pool.tile([P, KT, ncols], f16, tag="wloT")
                    if _rep == 0 and nval_c < ncols:
                        # first use of the slot: zero the never-gathered tail
                        # columns so downstream matmuls read finite data
                        nc.vector.memset(whiT[:, :, nval_c:], 0)
                        nc.vector.memset(wloT[:, :, nval_c:], 0)
                    nc.gpsimd.dma_gather(
                        out_ap=whiT[:],
                        in_ap=whi_dram,
                        idxs_ap=idx_sb[:, r0 // 16 : (r0 + ncols) // 16],
                        num_idxs=ncols,
                        num_idxs_reg=nval_c,
                        elem_size=D_MODEL,
                        transpose=True,
                    )
                    nc.gpsimd.dma_gather(
                        out_ap=wloT[:],
                        in_ap=wlo_dram,
                        idxs_ap=idx_sb[:, r0 // 16 : (r0 + ncols) // 16],
                        num_idxs=ncols,
                        num_idxs_reg=nval_c,
                        elem_size=D_MODEL,
                        transpose=True,
                    )
                elif mode == "dma_nt":
                    whiT = whi_pool.tile([P, -(-ncols // P), D_MODEL], f16, tag="whiT")
                    nc.gpsimd.dma_gather(
                        out_ap=whiT[:],
                        in_ap=whi_dram,
                        idxs_ap=idx_sb[:, r0 // 16 : (r0 + ncols) // 16],
                        num_idxs=ncols,
                        num_idxs_reg=ncols,
                        elem_size=D_MODEL,
                        transpose=False,
                    )
                    wloT = wlo_pool.tile([P, -(-ncols // P), D_MODEL], f16, tag="wloT")
                    nc.gpsimd.dma_gather(
                        out_ap=wloT[:],
                        in_ap=wlo_dram,
                        idxs_ap=idx_sb[:, r0 // 16 : (r0 + ncols) // 16],
                        num_idxs=ncols,
                        num_idxs_reg=ncols,
                        elem_size=D_MODEL,
                        transpose=False,
                    )
                else:
                    whiT = whiT_c
                    wloT = wloT_c

                if mode in ("dma", "dma_nt"):
                    continue

                if mode in ("fused", "fused2"):
                    # One PSUM chain: rows 0-31 accumulate xh*whi (hi chain);
                    # rows 32-63 accumulate xl_s*whi (mm1) AND xh*wlo_s (mm2).
                    # The group is opened by mm1@k=0 (spans rows 0-63) and
                    # closed by mm1@k=31, so mm2@k=31 is emitted before it.
                    psAB = psum.tile([64, ncols], mybir.dt.float32, tag="psA")

                    def mm1(k):
                        nc.tensor.matmul(
                            out=psAB[:],
                            lhsT=xp_sb[:, k * 2 * B : (k + 1) * 2 * B],
                            rhs=whiT[:, k, :],
                            start=(k == 0),
                            stop=(k == KT - 1),
                        )

                    def mm2(k):
                        nc.tensor.matmul(
                            out=psAB[B : 2 * B, :],
                            lhsT=xp_sb[:, k * 2 * B : k * 2 * B + B],
                            rhs=wloT[:, k, :],
                            start=False,
                            stop=False,
                        )

                    # mm1s first: they only depend on the whi gather, so the
                    # PE starts before wlo lands. mm1@KT-1 closes the group.
                    for k in range(KT - 1):
                        mm1(k)
                    for k in range(KT):
                        mm2(k)
                    mm1(KT - 1)
                    if mode == "fused2":
                        # recombine on the PE: out = combM.T @ [hi; lo; 0]
                        # = hi + lo/2048, landing directly on partitions 0-31
                        cmb_sb = out_pool.tile([P, ncols], f32, tag="cmb")
                        if _rep == 0 and c < 2:
                            nc.vector.memset(cmb_sb[2 * B :, :], 0)
                        nc.vector.tensor_copy(cmb_sb[: 2 * B, :], psAB[:])
                        psO = psum.tile([B, ncols], mybir.dt.float32, tag="psO")
                        nc.tensor.matmul(
                            out=psO[:], lhsT=combM[:], rhs=cmb_sb[:],
                            start=True, stop=True,
                        )
                        nc.scalar.copy(out_sb[:, r0 : r0 + ncols], psO[:])
                        if not tiny_out:
                            nc.sync.dma_start(
                                out_dram[:, r0 : r0 + ncols],
                                out_sb[:, r0 : r0 + ncols],
                            )
                        continue
                    # hi chain copied to partitions 0-31; lo-sum scaled on
                    # 32-63, realigned to 0-31 with an HWDGE SBUF-SBUF DMA
                    # (keeps the tail off the SWDGE queue the gathers use),
                    # then added and written out per chunk.
                    nc.scalar.copy(outA_sb[:, r0 : r0 + ncols], psAB[:B, :])
                    nc.vector.tensor_scalar_mul(
                        t1_sb[B : 2 * B, r0 : r0 + ncols],
                        psAB[B : 2 * B, :],
                        1.0 / LO_SCALE,
                    )
                    nc.sync.dma_start(
                        out_sb[:, r0 : r0 + ncols],
                        t1_sb[B : 2 * B, r0 : r0 + ncols],
                    )
                    nc.vector.tensor_add(
                        out_sb[:, r0 : r0 + ncols],
                        out_sb[:, r0 : r0 + ncols],
                        outA_sb[:, r0 : r0 + ncols],
                    )
                    if not tiny_out:
                        nc.sync.dma_start(
                            out_dram[:, r0 : r0 + ncols],
                            out_sb[:, r0 : r0 + ncols],
                        )
                    continue

                # mode full/mm: 3 matmul passes, both chains on partitions 0-31
                mcols = 256 if mode == "mm" else ncols
                psA = psum.tile([B, mcols], mybir.dt.float32, tag="psA")
                psB = psum.tile([B, mcols], mybir.dt.float32, tag="psB")
                for k in range(KT):
                    xh_k = xh_sb[:, k * B : (k + 1) * B]
                    xl_k = xl_sb[:, k * B : (k + 1) * B]
                    nc.tensor.matmul(
                        out=psA[:],
                        lhsT=xh_k,
                        rhs=whiT[:, k, :mcols],
                        start=(k == 0),
                        stop=(k == KT - 1),
                    )
                    nc.tensor.matmul(
                        out=psB[:],
                        lhsT=xh_k,
                        rhs=wloT[:, k, :mcols],
                        start=(k == 0),
                        stop=False,
                    )
                    nc.tensor.matmul(
                        out=psB[:],
                        lhsT=xl_k,
                        rhs=whiT[:, k, :mcols],
                        start=False,
                        stop=(k == KT - 1),
                    )
                dst = out_sb[:, r0 : r0 + ncols]
                nc.scalar.mul(dst, psB[:, :ncols], 1.0 / LO_SCALE)
                nc.vector.tensor_add(dst, dst, psA[:, :ncols])

            if mode == "fused":
                if tiny_out:
                    nc.sync.dma_start(out_dram, outA_sb[:, :out_cols])
                continue
            if mode == "fused2":
                continue
            if mode in ("dma", "dma_nt"):
                nc.vector.tensor_copy(out_sb[:, :64], whiT[:32, 0, :64])
            nc.sync.dma_start(out_dram, out_sb[:, :out_cols])

    nc.compile()
    _cache[key] = nc
    return nc


def _prep_scatter(nc, out_dram, obs, idx_sb, nv16, c):
    B_ = B
    nc.gpsimd.dma_scatter_add(
        out_ap=out_dram,
        in_ap=obs[:, c : c + 1, :],
        idxs_ap=idx_sb[:, nv16 + c * 8 : nv16 + c * 8 + 8],
        num_idxs=128,
        num_idxs_reg=128,
        elem_size=2 * B_,
        prepare_only=True,
        queue_num=1,
    )


def _build_hi(nvalid, gbufs=3, reps=1):
    """fp16-only weight-stationary kernel.

    One fp16 gather per 128-row chunk lands matmul-ready [d%128, k, r].
    Matmuls are weight-stationary: lhsT = gathered chunk [128, ncols<=128],
    rhs = x^T fp16 [128, 32] per contraction tile, accumulating
    psT[r, b] over the 32 k-tiles. Output is written transposed
    [nvalid, B]; the host transposes during assembly. fp16 on both sides
    gives ~2.4e-4 scale-rel error (gate is 2e-2).
    """
    key = ("hi3", nvalid, gbufs, reps)
    if key in _cache:
        return _cache[key]
    from concourse import bacc, mybir, tile

    f32 = mybir.dt.float32
    f16 = mybir.dt.float16
    i16 = mybir.dt.int16

    nfull = nvalid // 128      # full 128-row transpose-gather chunks
    nt = nvalid % 128          # odd-size tail chunk, non-transpose gather
    nch = nfull + (1 if nt else 0)
    nv16 = nvalid // 16

    nc = bacc.Bacc(
        "TRN2",
        target_bir_lowering=False,
        debug=False,
        enable_asserts=False,
        num_swdge_queues=2,
    )
    whi_dram = nc.dram_tensor("whi", [D_FF, D_MODEL], f16, kind="ExternalInput").ap()
    xh_dram = nc.dram_tensor("xh", [P, KT * B], f16, kind="ExternalInput").ap()
    if nt:
        id_dram = nc.dram_tensor("ident", [nt, nt], f16, kind="ExternalInput").ap()
    # first nv16 cols: gather indices; next nv16: iota rows for the scatter
    idx_dram = nc.dram_tensor("idx", [P, 2 * nv16], i16, kind="ExternalInput").ap()
    # 64-wide rows so the scatter elem is 256B (cols 32..63 are zero pad)
    out_dram = nc.dram_tensor("out", [nvalid, 2 * B], f32, kind="ExternalOutput").ap()

    with tile.TileContext(nc) as tc, ExitStack() as ctx:
        consts = ctx.enter_context(tc.tile_pool(name="consts", bufs=1))
        whi_pool = ctx.enter_context(tc.tile_pool(name="whiT", bufs=max(gbufs, nch)))
        psum = ctx.enter_context(tc.tile_pool(name="psum", bufs=4, space="PSUM"))

        idx_sb = consts.tile([P, 2 * nv16], i16)
        nc.sync.dma_start(idx_sb[:], idx_dram)
        xh_sb = consts.tile([P, KT * B], f16)
        nc.sync.dma_start(xh_sb[:], xh_dram)
        obs = consts.tile([P, nch, 2 * B], f32)
        nc.vector.memset(obs[:], 0)
        if nt:
            id_sb = consts.tile([nt, nt], f16)
            nc.sync.dma_start(id_sb[:], id_dram)
            # partitions >= nt are never written or read: no memset needed
            w4 = consts.tile([P, 1, D_MODEL], f16)

        # issue all gathers + output-scatter preps first so the Pool SEQ is
        # never blocked behind a trigger's data wait; desc-gen pipelines
        # ahead of the serialized DMA transfers
        whiTs = []
        if nt:
            # odd tail gathered FIRST (its transfer is the shortest and its
            # PE transposes hide under the remaining gathers' transfers)
            nc.gpsimd.dma_gather(
                out_ap=w4[:],
                in_ap=whi_dram,
                idxs_ap=idx_sb[:, nfull * 8 : nv16],
                num_idxs=nt,
                num_idxs_reg=nt,
                elem_size=D_MODEL,
                transpose=False,
            )
        for c in range(nfull):
            r0 = c * 128
            whiT = whi_pool.tile([P, KT, 128], f16, tag=f"whiT{c}")
            nc.gpsimd.dma_gather(
                out_ap=whiT[:],
                in_ap=whi_dram,
                idxs_ap=idx_sb[:, r0 // 16 : r0 // 16 + 8],
                num_idxs=128,
                num_idxs_reg=128,
                elem_size=D_MODEL,
                transpose=True,
            )
            whiTs.append(whiT)
        if nt:
            # on-chip transpose of the tail rows into matmul-ready layout
            whiT_nt = whi_pool.tile([P, KT, nt], f16, tag="whiTnt")
            for k in range(KT):
                psX = psum.tile([P, nt], f16, tag="psX")
                nc.tensor.transpose(
                    psX[:], w4[:nt, 0, k * P : (k + 1) * P], id_sb[:]
                )
                nc.vector.tensor_copy(whiT_nt[:, k, :], psX[:])
            whiTs.append(whiT_nt)

        order = list(range(nch))
        if nt and nch >= 2:
            # whiTs[nch-1] is the nt chunk (data ready early); run it before
            # the last full chunk, whose gather finishes last
            order = order[: nch - 2] + [nch - 1, nch - 2]
        for c in order:
            whiT = whiTs[c]
            rows = nt if (nt and c == nch - 1) else 128
            r0 = nfull * 128 if (nt and c == nch - 1) else c * 128
            psT = psum.tile([rows, B], f32, tag="psT")
            for k in range(KT):
                nc.tensor.matmul(
                    out=psT[:],
                    lhsT=whiT[:, k, :],
                    rhs=xh_sb[:, k * B : (k + 1) * B],
                    start=(k == 0),
                    stop=(k == KT - 1),
                )
            nc.scalar.copy(obs[:rows, c, :B], psT[:])
            nc.sync.dma_start(out_dram[r0 : r0 + rows, :B], obs[:rows, c, :B])

    nc.compile()
    _cache[key] = nc
    return nc


def _make_in_maps_hi(x, weight, indices):
    """Host prep for the hi kernel: dedup+shard indices, fp16 casts.

    Returns (in_maps, assemble_fn, nvalid)."""
    x = np.asarray(x, dtype=np.float32)
    weight = np.asarray(weight, dtype=np.float32)
    indices = np.asarray(indices, dtype=np.int64)

    whi = np.ascontiguousarray(weight.astype(np.float16))
    xt = np.ascontiguousarray(
        x[:, 0, :].reshape(B, KT, P).transpose(2, 1, 0).reshape(P, KT * B)
    )
    xh = np.ascontiguousarray(xt.astype(np.float16))

    uniq, inv = np.unique(indices, return_inverse=True)
    nu = len(uniq)
    base, rem = divmod(nu, NCORES)
    counts = [base + (1 if c < rem else 0) for c in range(NCORES)]
    starts = np.concatenate([[0], np.cumsum(counts)[:-1]])
    # multiples of 128 go through transpose gathers; the %16 tail is a
    # non-transpose gather + on-chip PE transpose
    nvalid = -(-max(counts) // 16) * 16

    iota = _wrap_idx(np.arange(nvalid, dtype=np.int16))
    ident = np.eye(nvalid % 128 or 1, dtype=np.float16)
    in_maps = []
    for c in range(NCORES):
        idx_pad = np.zeros(nvalid, dtype=np.int16)
        idx_pad[: counts[c]] = uniq[starts[c] : starts[c] + counts[c]]
        idx_full = np.concatenate([_wrap_idx(idx_pad), iota], axis=1)
        in_maps.append(
            {
                "whi": whi,
                "xh": xh,
                "idx": np.ascontiguousarray(idx_full),
                "ident": ident,
            }
        )

    def assemble(results):
        cols = np.empty((B, nu), dtype=np.float32)
        for c in range(NCORES):
            cols[:, starts[c] : starts[c] + counts[c]] = (
                results[c]["out"][: counts[c], :B].T
            )
        return np.ascontiguousarray(cols[:, inv].reshape(B, 1, R_TOTAL))

    return in_maps, assemble, nvalid


def _split_pair(a):
    """fp32 -> (hi fp16, (a-hi)*2048 fp16). hi + lo/2048 ~= a to ~2^-22 rel."""
    hi = a.astype(np.float16)
    lo = ((a - hi.astype(np.float32)) * LO_SCALE).astype(np.float16)
    return hi, lo


def _wrap_idx(idx_pad):
    """[npad] int16 -> [128, npad//16] wrapped-16 layout, replicated 8x."""
    npad = idx_pad.shape[0]
    blk = idx_pad.reshape(npad // 16, 16).T  # [16, npad//16]
    return np.ascontiguousarray(np.tile(blk, (8, 1)))


def _make_in_maps(x, weight, indices, dedup=True):
    """Returns (in_maps, assemble_fn, npad)."""
    x = np.asarray(x, dtype=np.float32)
    weight = np.ascontiguousarray(np.asarray(weight, dtype=np.float32))
    indices = np.asarray(indices, dtype=np.int64)

    whi, wlo = _split_pair(weight)
    whi = np.ascontiguousarray(whi)
    wlo = np.ascontiguousarray(wlo)

    # x^T staged so the DMA is contiguous: xt[p, k*32+b] = x[b, 0, k*128+p]
    xt = np.ascontiguousarray(
        x[:, 0, :].reshape(B, KT, P).transpose(2, 1, 0).reshape(P, KT * B)
    )
    xh, xl = _split_pair(xt)
    # packed [xh || xl] per contraction tile for the fused M=64 matmul
    xp = np.empty((P, KT, 2 * B), dtype=np.float16)
    xp[:, :, :B] = xh.reshape(P, KT, B)
    xp[:, :, B:] = xl.reshape(P, KT, B)
    xp = np.ascontiguousarray(xp.reshape(P, KT * 2 * B))
    combm = np.zeros((P, B), dtype=np.float32)
    combm[:B, :] = np.eye(B, dtype=np.float32)
    combm[B : 2 * B, :] = np.eye(B, dtype=np.float32) / LO_SCALE

    uniq, inv = np.unique(indices, return_inverse=True)
    nu = len(uniq)
    use_dedup = dedup and -(-nu // NCORES) <= NPAD_DEDUP
    if use_dedup:
        npad = NPAD_DEDUP
        base, rem = divmod(nu, NCORES)
        counts = [base + (1 if c < rem else 0) for c in range(NCORES)]
        starts = np.concatenate([[0], np.cumsum(counts)[:-1]])
        core_idx = [uniq[starts[c] : starts[c] + counts[c]] for c in range(NCORES)]
    else:
        npad = NPAD
        counts = list(_CORE_N)
        starts = list(_CORE_START)
        core_idx = [
            indices[starts[c] : starts[c] + counts[c]] for c in range(NCORES)
        ]

    # uniform valid count (dup-padded with row 0); -1 beyond it is trimmed
    # from the gather transfer by the SWDGE
    nvalid = min(-(-max(counts) // 16) * 16, npad)

    in_maps = []
    for c in range(NCORES):
        idx_pad = np.full(npad, -1, dtype=np.int16)
        idx_pad[: counts[c]] = core_idx[c]
        idx_pad[counts[c] : nvalid] = 0
        in_maps.append(
            {
                "whi": whi,
                "wlo": wlo,
                "xh": np.ascontiguousarray(xh),
                "xl": np.ascontiguousarray(xl),
                "xp": xp,
                "combm": combm,
                "idx": _wrap_idx(idx_pad),
            }
        )

    def assemble(results):
        cols = np.empty((B, sum(counts)), dtype=np.float32)
        for c in range(NCORES):
            cols[:, starts[c] : starts[c] + counts[c]] = results[c]["out"][
                :, : counts[c]
            ]
        if use_dedup:
            out = cols[:, inv]
        else:
            out = cols
        return np.ascontiguousarray(out.reshape(B, 1, R_TOTAL))

    return in_maps, assemble, npad, nvalid


def _filter_in_maps(nc, in_maps):
    names = set()
    from concourse import mybir

    for alloc in nc.m.functions[0].allocations:
        if isinstance(alloc, mybir.MemoryLocationSet) and alloc.kind == "ExternalInput":
            names.add(alloc.memorylocations[0].name)
    return [{k: v for k, v in m.items() if k in names} for m in in_maps]


def run_full(x, weight, indices, trace=False, mode="hi", dedup=True):
    """Returns (output, BassKernelResults)."""
    from concourse.bass_utils import run_bass_kernel_spmd

    if mode == "hi":
        in_maps, assemble, nvalid = _make_in_maps_hi(x, weight, indices)
        nc = _build_hi(nvalid)
        in_maps = _filter_in_maps(nc, in_maps)
        res = run_bass_kernel_spmd(nc, in_maps, list(range(NCORES)), trace=trace)
        return assemble(res.results), res

    in_maps, assemble, npad, nvalid = _make_in_maps(x, weight, indices, dedup=dedup)
    if mode == "fused2" and npad != NPAD_DEDUP:
        # the fused2 epilogue is only validated for the 512-wide dedup
        # layout; the rare >4096-unique fallback uses the fused tail
        mode = "fused"
    nc = _build(1, mode, False, npad, nvalid=nvalid)
    in_maps = _filter_in_maps(nc, in_maps)
    res = run_bass_kernel_spmd(nc, in_maps, list(range(NCORES)), trace=trace)
    return assemble(res.results), res


def kernel(x, weight, indices):
    out, _ = run_full(x, weight, indices)
    return out



# revision 23
# speedup vs baseline: 1.8711x; 1.0106x over previous
"""Trainium2 Bass kernel for gathered-row MLP decode matmul.

out[b, 0, r] = sum_d x[b, 0, d] * weight[indices[r], d]

Active path (kernel() -> run_full(mode="hi") -> _build_hi): dedup+sort the
indices on the host and shard them across 8 cores (~452 rows each). The
weight is cast to fp16 on the host (one 2-byte copy instead of an fp32 or
hi/lo pair: halves both HBM traffic and PE work; end-to-end scale-rel
error ~2.4e-4 against the 2e-2 gate). Each core gathers its rows in
128-row transpose-gather chunks that land matmul-ready [d%128, k, r];
the %128 tail chunk uses a non-transpose gather plus PE transposes that
hide under the other chunks' DMA transfers. Matmuls are weight-stationary
(lhsT = gathered [128, rows] tile, moving = x^T fp16 [128, 32] per
contraction tile), accumulating out^T [rows, 32] in PSUM; each chunk is
copied to SBUF and DMAed out transposed. The host transposes/assembles
per-core outputs and inverse-maps duplicates back to the original 4403
index order. The older fp32-accurate hi/lo modes are kept in _build()
for reference/fallback.
"""
import os
import sys
from contextlib import ExitStack

sys.path.insert(0, "/opt/trn_rl_repo")
os.environ.setdefault("MYCRO_LOCAL_CACHE", "1")

import numpy as np

D_FF = 11008
D_MODEL = 4096
R_TOTAL = 4403
B = 32
NCORES = 8
P = 128
KT = D_MODEL // P          # 32 contraction tiles
NPAD = 640                 # padded per-core index count (5*128), fallback
NPAD_DEDUP = 512           # padded per-core count for the dedup path
LO_SCALE = 2048.0          # wlo/xlo pre-scale (2^11)

# per-core share of the real 4403 indices (no-dedup fallback)
_CORE_N = [551, 551, 551, 550, 550, 550, 550, 550]
_CORE_START = [0]
for _n in _CORE_N[:-1]:
    _CORE_START.append(_CORE_START[-1] + _n)

_cache = {}


def _build(
    reps=1, mode="full", tiny_out=False, npad=NPAD, chunks=None, gbufs=2,
    nvalid=None,
):
    """mode: full (3-matmul) | fused (2-pass M=64 packing) | dma (gathers
    only) | dma_nt (non-transpose gathers) | mm (matmuls only).
    tiny_out: shrink the DRAM output to [B, 64] so bench-loop host
    transfers are negligible (timing only)."""
    if nvalid is None:
        nvalid = npad
    key = ("nc", reps, mode, tiny_out, npad, chunks, gbufs, nvalid)
    if key in _cache:
        return _cache[key]
    from concourse import bacc, mybir, tile

    f32 = mybir.dt.float32
    f16 = mybir.dt.float16
    i16 = mybir.dt.int16

    if chunks is None:
        chunks = tuple((i, min(256, npad - i)) for i in range(0, npad, 256))

    nc = bacc.Bacc(
        "TRN2", target_bir_lowering=False, debug=False, enable_asserts=False
    )
    whi_dram = nc.dram_tensor("whi", [D_FF, D_MODEL], f16, kind="ExternalInput").ap()
    wlo_dram = nc.dram_tensor("wlo", [D_FF, D_MODEL], f16, kind="ExternalInput").ap()
    if mode in ("fused", "fused2"):
        xp_dram = nc.dram_tensor("xp", [P, KT * 2 * B], f16, kind="ExternalInput").ap()
        if mode == "fused2":
            combm_dram = nc.dram_tensor(
                "combm", [P, B], f32, kind="ExternalInput"
            ).ap()
    else:
        xh_dram = nc.dram_tensor("xh", [P, KT * B], f16, kind="ExternalInput").ap()
        xl_dram = nc.dram_tensor("xl", [P, KT * B], f16, kind="ExternalInput").ap()
    idx_dram = nc.dram_tensor("idx", [P, npad // 16], i16, kind="ExternalInput").ap()
    out_cols = 64 if tiny_out else npad
    out_dram = nc.dram_tensor("out", [B, out_cols], f32, kind="ExternalOutput").ap()

    with tile.TileContext(nc) as tc, ExitStack() as ctx:
        consts = ctx.enter_context(tc.tile_pool(name="consts", bufs=1))
        whi_pool = ctx.enter_context(tc.tile_pool(name="whiT", bufs=gbufs))
        wlo_pool = ctx.enter_context(tc.tile_pool(name="wloT", bufs=gbufs))
        psum = ctx.enter_context(tc.tile_pool(name="psum", bufs=4, space="PSUM"))
        out_pool = ctx.enter_context(tc.tile_pool(name="outp", bufs=2))

        # idx first: the gathers (the critical path) depend only on it
        idx_sb = consts.tile([P, npad // 16], i16)
        nc.sync.dma_start(idx_sb[:], idx_dram)
        if mode in ("fused", "fused2"):
            xp_sb = consts.tile([P, KT * 2 * B], f16)
            nc.sync.dma_start(xp_sb[:], xp_dram)
        else:
            xh_sb = consts.tile([P, KT * B], f16)
            nc.sync.dma_start(xh_sb[:], xh_dram)
            xl_sb = consts.tile([P, KT * B], f16)
            nc.sync.dma_start(xl_sb[:], xl_dram)

        if mode == "mm":
            whiT_c = consts.tile([P, KT, 256], f16)
            nc.gpsimd.memset(whiT_c[:], 0.25)
            wloT_c = consts.tile([P, KT, 256], f16)
            nc.gpsimd.memset(wloT_c[:], 0.25)

        if mode == "fused2":
            # combM.T @ [hi; lo; 0] = hi + lo/2048 (host-built constant;
            # K padded to 128 so the fp32 matmul uses the proven full
            # partition-group shape)
            combM = consts.tile([P, B], f32)
            nc.sync.dma_start(combM[:], combm_dram)

        for _rep in range(reps):
            out_sb = out_pool.tile([B, npad], f32, tag="out_sb")
            if mode == "fused":
                t1_sb = out_pool.tile([64, npad], f32, tag="t1")
                outA_sb = out_pool.tile([B, npad], f32, tag="outA")

            for c, (r0, ncols) in enumerate(chunks):
                if mode in ("full", "fused", "fused2", "dma"):
                    # valid (non -1) indices in this chunk; the SWDGE trims
                    # the transfer to the valid prefix, so -1 tail padding
                    # costs no DMA.
                    nval_c = max(0, min(nvalid - r0, ncols))
                    # whiT[p, k, i] = whi[idx[r0+i], k*128 + p]
                    whiT = whi_pool.tile([P, KT, ncols], f16, tag="whiT")
                    wloT = wlo_pool.tile([P, KT, ncols], f16, tag="wloT")
                    if _rep == 0 and nval_c < ncols:
                        # first use of the slot: zero the never-gathered tail
                        # columns so downstream matmuls read finite data
                        nc.vector.memset(whiT[:, :, nval_c:], 0)
                        nc.vector.memset(wloT[:, :, nval_c:], 0)
                    nc.gpsimd.dma_gather(
                        out_ap=whiT[:],
                        in_ap=whi_dram,
                        idxs_ap=idx_sb[:, r0 // 16 : (r0 + ncols) // 16],
                        num_idxs=ncols,
                        num_idxs_reg=nval_c,
                        elem_size=D_MODEL,
                        transpose=True,
                    )
                    nc.gpsimd.dma_gather(
                        out_ap=wloT[:],
                        in_ap=wlo_dram,
                        idxs_ap=idx_sb[:, r0 // 16 : (r0 + ncols) // 16],
                        num_idxs=ncols,
                        num_idxs_reg=nval_c,
                        elem_size=D_MODEL,
                        transpose=True,
                    )
                elif mode == "dma_nt":
                    whiT = whi_pool.tile([P, -(-ncols // P), D_MODEL], f16, tag="whiT")
                    nc.gpsimd.dma_gather(
                        out_ap=whiT[:],
                        in_ap=whi_dram,
                        idxs_ap=idx_sb[:, r0 // 16 : (r0 + ncols) // 16],
                        num_idxs=ncols,
                        num_idxs_reg=ncols,
                        elem_size=D_MODEL,
                        transpose=False,
                    )
                    wloT = wlo_pool.tile([P, -(-ncols // P), D_MODEL], f16, tag="wloT")
                    nc.gpsimd.dma_gather(
                        out_ap=wloT[:],
                        in_ap=wlo_dram,
                        idxs_ap=idx_sb[:, r0 // 16 : (r0 + ncols) // 16],
                        num_idxs=ncols,
                        num_idxs_reg=ncols,
                        elem_size=D_MODEL,
                        transpose=False,
                    )
                else:
                    whiT = whiT_c
                    wloT = wloT_c

                if mode in ("dma", "dma_nt"):
                    continue

                if mode in ("fused", "fused2"):
                    # One PSUM chain: rows 0-31 accumulate xh*whi (hi chain);
                    # rows 32-63 accumulate xl_s*whi (mm1) AND xh*wlo_s (mm2).
                    # The group is opened by mm1@k=0 (spans rows 0-63) and
                    # closed by mm1@k=31, so mm2@k=31 is emitted before it.
                    psAB = psum.tile([64, ncols], mybir.dt.float32, tag="psA")

                    def mm1(k):
                        nc.tensor.matmul(
                            out=psAB[:],
                            lhsT=xp_sb[:, k * 2 * B : (k + 1) * 2 * B],
                            rhs=whiT[:, k, :],
                            start=(k == 0),
                            stop=(k == KT - 1),
                        )

                    def mm2(k):
                        nc.tensor.matmul(
                            out=psAB[B : 2 * B, :],
                            lhsT=xp_sb[:, k * 2 * B : k * 2 * B + B],
                            rhs=wloT[:, k, :],
                            start=False,
                            stop=False,
                        )

                    # mm1s first: they only depend on the whi gather, so the
                    # PE starts before wlo lands. mm1@KT-1 closes the group.
                    for k in range(KT - 1):
                        mm1(k)
                    for k in range(KT):
                        mm2(k)
                    mm1(KT - 1)
                    if mode == "fused2":
                        # recombine on the PE: out = combM.T @ [hi; lo; 0]
                        # = hi + lo/2048, landing directly on partitions 0-31
                        cmb_sb = out_pool.tile([P, ncols], f32, tag="cmb")
                        if _rep == 0 and c < 2:
                            nc.vector.memset(cmb_sb[2 * B :, :], 0)
                        nc.vector.tensor_copy(cmb_sb[: 2 * B, :], psAB[:])
                        psO = psum.tile([B, ncols], mybir.dt.float32, tag="psO")
                        nc.tensor.matmul(
                            out=psO[:], lhsT=combM[:], rhs=cmb_sb[:],
                            start=True, stop=True,
                        )
                        nc.scalar.copy(out_sb[:, r0 : r0 + ncols], psO[:])
                        if not tiny_out:
                            nc.sync.dma_start(
                                out_dram[:, r0 : r0 + ncols],
                                out_sb[:, r0 : r0 + ncols],
                            )
                        continue
                    # hi chain copied to partitions 0-31; lo-sum scaled on
                    # 32-63, realigned to 0-31 with an HWDGE SBUF-SBUF DMA
                    # (keeps the tail off the SWDGE queue the gathers use),
                    # then added and written out per chunk.
                    nc.scalar.copy(outA_sb[:, r0 : r0 + ncols], psAB[:B, :])
                    nc.vector.tensor_scalar_mul(
                        t1_sb[B : 2 * B, r0 : r0 + ncols],
                        psAB[B : 2 * B, :],
                        1.0 / LO_SCALE,
                    )
                    nc.sync.dma_start(
                        out_sb[:, r0 : r0 + ncols],
                        t1_sb[B : 2 * B, r0 : r0 + ncols],
                    )
                    nc.vector.tensor_add(
                        out_sb[:, r0 : r0 + ncols],
                        out_sb[:, r0 : r0 + ncols],
                        outA_sb[:, r0 : r0 + ncols],
                    )
                    if not tiny_out:
                        nc.sync.dma_start(
                            out_dram[:, r0 : r0 + ncols],
                            out_sb[:, r0 : r0 + ncols],
                        )
                    continue

                # mode full/mm: 3 matmul passes, both chains on partitions 0-31
                mcols = 256 if mode == "mm" else ncols
                psA = psum.tile([B, mcols], mybir.dt.float32, tag="psA")
                psB = psum.tile([B, mcols], mybir.dt.float32, tag="psB")
                for k in range(KT):
                    xh_k = xh_sb[:, k * B : (k + 1) * B]
                    xl_k = xl_sb[:, k * B : (k + 1) * B]
                    nc.tensor.matmul(
                        out=psA[:],
                        lhsT=xh_k,
                        rhs=whiT[:, k, :mcols],
                        start=(k == 0),
                        stop=(k == KT - 1),
                    )
                    nc.tensor.matmul(
                        out=psB[:],
                        lhsT=xh_k,
                        rhs=wloT[:, k, :mcols],
                        start=(k == 0),
                        stop=False,
                    )
                    nc.tensor.matmul(
                        out=psB[:],
                        lhsT=xl_k,
                        rhs=whiT[:, k, :mcols],
                        start=False,
                        stop=(k == KT - 1),
                    )
                dst = out_sb[:, r0 : r0 + ncols]
                nc.scalar.mul(dst, psB[:, :ncols], 1.0 / LO_SCALE)
                nc.vector.tensor_add(dst, dst, psA[:, :ncols])

            if mode == "fused":
                if tiny_out:
                    nc.sync.dma_start(out_dram, outA_sb[:, :out_cols])
                continue
            if mode == "fused2":
                continue
            if mode in ("dma", "dma_nt"):
                nc.vector.tensor_copy(out_sb[:, :64], whiT[:32, 0, :64])
            nc.sync.dma_start(out_dram, out_sb[:, :out_cols])

    nc.compile()
    _cache[key] = nc
    return nc


def _prep_scatter(nc, out_dram, obs, idx_sb, nv16, c):
    B_ = B
    nc.gpsimd.dma_scatter_add(
        out_ap=out_dram,
        in_ap=obs[:, c : c + 1, :],
        idxs_ap=idx_sb[:, nv16 + c * 8 : nv16 + c * 8 + 8],
        num_idxs=128,
        num_idxs_reg=128,
        elem_size=2 * B_,
        prepare_only=True,
        queue_num=1,
    )


def _build_hi(nvalid, gbufs=3, reps=1):
    """fp16-only weight-stationary kernel.

    One fp16 gather per 128-row chunk lands matmul-ready [d%128, k, r].
    Matmuls are weight-stationary: lhsT = gathered chunk [128, ncols<=128],
    rhs = x^T fp16 [128, 32] per contraction tile, accumulating
    psT[r, b] over the 32 k-tiles. Output is written transposed
    [nvalid, B]; the host transposes during assembly. fp16 on both sides
    gives ~2.4e-4 scale-rel error (gate is 2e-2).
    """
    key = ("hi4", nvalid, gbufs, reps)
    if key in _cache:
        return _cache[key]
    from concourse import bacc, mybir, tile

    f32 = mybir.dt.float32
    f16 = mybir.dt.float16
    i16 = mybir.dt.int16

    nfull = nvalid // 128      # full 128-row transpose-gather chunks
    nt = nvalid % 128          # odd-size tail chunk, non-transpose gather
    nch = nfull + (1 if nt else 0)
    nv16 = nvalid // 16

    nc = bacc.Bacc(
        "TRN2",
        target_bir_lowering=False,
        debug=False,
        enable_asserts=False,
        num_swdge_queues=2,
    )
    whi_dram = nc.dram_tensor("whi", [D_FF, D_MODEL], f16, kind="ExternalInput").ap()
    xh_dram = nc.dram_tensor("xh", [P, KT * B], f16, kind="ExternalInput").ap()
    if nt:
        id_dram = nc.dram_tensor("ident", [nt, nt], f16, kind="ExternalInput").ap()
    # first nv16 cols: gather indices; next nv16: iota rows for the scatter
    idx_dram = nc.dram_tensor("idx", [P, 2 * nv16], i16, kind="ExternalInput").ap()
    # 64-wide rows so the scatter elem is 256B (cols 32..63 are zero pad)
    out_dram = nc.dram_tensor("out", [nvalid, 2 * B], f32, kind="ExternalOutput").ap()

    with tile.TileContext(nc) as tc, ExitStack() as ctx:
        consts = ctx.enter_context(tc.tile_pool(name="consts", bufs=1))
        whi_pool = ctx.enter_context(tc.tile_pool(name="whiT", bufs=max(gbufs, nch)))
        psum = ctx.enter_context(tc.tile_pool(name="psum", bufs=4, space="PSUM"))

        idx_sb = consts.tile([P, 2 * nv16], i16)
        nc.sync.dma_start(idx_sb[:], idx_dram)
        xh_sb = consts.tile([P, KT * B], f16)
        nc.sync.dma_start(xh_sb[:], xh_dram)
        obs = consts.tile([P, nch, 2 * B], f32)
        nc.vector.memset(obs[:], 0)
        if nt:
            id_sb = consts.tile([nt, nt], f16)
            nc.sync.dma_start(id_sb[:], id_dram)
            # partitions >= nt are never written or read: no memset needed
            w4 = consts.tile([P, 1, D_MODEL], f16)

        # issue all gathers + output-scatter preps first so the Pool SEQ is
        # never blocked behind a trigger's data wait; desc-gen pipelines
        # ahead of the serialized DMA transfers
        whiTs = []
        if nt:
            # odd tail gathered FIRST (its transfer is the shortest and its
            # PE transposes hide under the remaining gathers' transfers)
            nc.gpsimd.dma_gather(
                out_ap=w4[:],
                in_ap=whi_dram,
                idxs_ap=idx_sb[:, nfull * 8 : nv16],
                num_idxs=nt,
                num_idxs_reg=nt,
                elem_size=D_MODEL,
                transpose=False,
            )
        for c in range(nfull):
            r0 = c * 128
            whiT = whi_pool.tile([P, KT, 128], f16, tag=f"whiT{c}")
            if c == nfull - 1:
                # last chunk in two half-row gathers: the k<16 matmuls can
                # start while the second half is still in flight
                for h in range(2):
                    nc.gpsimd.dma_gather(
                        out_ap=whiT[:, h * 16 : (h + 1) * 16, :],
                        in_ap=whi_dram[:, h * 2048 : (h + 1) * 2048],
                        idxs_ap=idx_sb[:, r0 // 16 : r0 // 16 + 8],
                        num_idxs=128,
                        num_idxs_reg=128,
                        elem_size=2048,
                        elem_step=D_MODEL,
                        transpose=True,
                    )
            else:
                nc.gpsimd.dma_gather(
                    out_ap=whiT[:],
                    in_ap=whi_dram,
                    idxs_ap=idx_sb[:, r0 // 16 : r0 // 16 + 8],
                    num_idxs=128,
                    num_idxs_reg=128,
                    elem_size=D_MODEL,
                    transpose=True,
                )
            whiTs.append(whiT)
        if nt:
            # on-chip transpose of the tail rows into matmul-ready layout
            whiT_nt = whi_pool.tile([P, KT, nt], f16, tag="whiTnt")
            for k in range(KT):
                psX = psum.tile([P, nt], f16, tag="psX")
                nc.tensor.transpose(
                    psX[:], w4[:nt, 0, k * P : (k + 1) * P], id_sb[:]
                )
                nc.vector.tensor_copy(whiT_nt[:, k, :], psX[:])
            whiTs.append(whiT_nt)

        order = list(range(nch))
        if nt and nch >= 2:
            # whiTs[nch-1] is the nt chunk (data ready early); run it before
            # the last full chunk, whose gather finishes last
            order = order[: nch - 2] + [nch - 1, nch - 2]
        for c in order:
            whiT = whiTs[c]
            rows = nt if (nt and c == nch - 1) else 128
            r0 = nfull * 128 if (nt and c == nch - 1) else c * 128
            psT = psum.tile([rows, B], f32, tag="psT")
            for k in range(KT):
                nc.tensor.matmul(
                    out=psT[:],
                    lhsT=whiT[:, k, :],
                    rhs=xh_sb[:, k * B : (k + 1) * B],
                    start=(k == 0),
                    stop=(k == KT - 1),
                )
            nc.scalar.copy(obs[:rows, c, :B], psT[:])
            nc.sync.dma_start(out_dram[r0 : r0 + rows, :B], obs[:rows, c, :B])

    nc.compile()
    _cache[key] = nc
    return nc


def _make_in_maps_hi(x, weight, indices):
    """Host prep for the hi kernel: dedup+shard indices, fp16 casts.

    Returns (in_maps, assemble_fn, nvalid)."""
    x = np.asarray(x, dtype=np.float32)
    weight = np.asarray(weight, dtype=np.float32)
    indices = np.asarray(indices, dtype=np.int64)

    whi = np.ascontiguousarray(weight.astype(np.float16))
    xt = np.ascontiguousarray(
        x[:, 0, :].reshape(B, KT, P).transpose(2, 1, 0).reshape(P, KT * B)
    )
    xh = np.ascontiguousarray(xt.astype(np.float16))

    uniq, inv = np.unique(indices, return_inverse=True)
    nu = len(uniq)
    base, rem = divmod(nu, NCORES)
    counts = [base + (1 if c < rem else 0) for c in range(NCORES)]
    starts = np.concatenate([[0], np.cumsum(counts)[:-1]])
    # multiples of 128 go through transpose gathers; the %16 tail is a
    # non-transpose gather + on-chip PE transpose
    nvalid = -(-max(counts) // 16) * 16

    iota = _wrap_idx(np.arange(nvalid, dtype=np.int16))
    ident = np.eye(nvalid % 128 or 1, dtype=np.float16)
    in_maps = []
    for c in range(NCORES):
        idx_pad = np.zeros(nvalid, dtype=np.int16)
        idx_pad[: counts[c]] = uniq[starts[c] : starts[c] + counts[c]]
        idx_full = np.concatenate([_wrap_idx(idx_pad), iota], axis=1)
        in_maps.append(
            {
                "whi": whi,
                "xh": xh,
                "idx": np.ascontiguousarray(idx_full),
                "ident": ident,
            }
        )

    def assemble(results):
        cols = np.empty((B, nu), dtype=np.float32)
        for c in range(NCORES):
            cols[:, starts[c] : starts[c] + counts[c]] = (
                results[c]["out"][: counts[c], :B].T
            )
        return np.ascontiguousarray(cols[:, inv].reshape(B, 1, R_TOTAL))

    return in_maps, assemble, nvalid


def _split_pair(a):
    """fp32 -> (hi fp16, (a-hi)*2048 fp16). hi + lo/2048 ~= a to ~2^-22 rel."""
    hi = a.astype(np.float16)
    lo = ((a - hi.astype(np.float32)) * LO_SCALE).astype(np.float16)
    return hi, lo


def _wrap_idx(idx_pad):
    """[npad] int16 -> [128, npad//16] wrapped-16 layout, replicated 8x."""
    npad = idx_pad.shape[0]
    blk = idx_pad.reshape(npad // 16, 16).T  # [16, npad//16]
    return np.ascontiguousarray(np.tile(blk, (8, 1)))


def _make_in_maps(x, weight, indices, dedup=True):
    """Returns (in_maps, assemble_fn, npad)."""
    x = np.asarray(x, dtype=np.float32)
    weight = np.ascontiguousarray(np.asarray(weight, dtype=np.float32))
    indices = np.asarray(indices, dtype=np.int64)

    whi, wlo = _split_pair(weight)
    whi = np.ascontiguousarray(whi)
    wlo = np.ascontiguousarray(wlo)

    # x^T staged so the DMA is contiguous: xt[p, k*32+b] = x[b, 0, k*128+p]
    xt = np.ascontiguousarray(
        x[:, 0, :].reshape(B, KT, P).transpose(2, 1, 0).reshape(P, KT * B)
    )
    xh, xl = _split_pair(xt)
    # packed [xh || xl] per contraction tile for the fused M=64 matmul
    xp = np.empty((P, KT, 2 * B), dtype=np.float16)
    xp[:, :, :B] = xh.reshape(P, KT, B)
    xp[:, :, B:] = xl.reshape(P, KT, B)
    xp = np.ascontiguousarray(xp.reshape(P, KT * 2 * B))
    combm = np.zeros((P, B), dtype=np.float32)
    combm[:B, :] = np.eye(B, dtype=np.float32)
    combm[B : 2 * B, :] = np.eye(B, dtype=np.float32) / LO_SCALE

    uniq, inv = np.unique(indices, return_inverse=True)
    nu = len(uniq)
    use_dedup = dedup and -(-nu // NCORES) <= NPAD_DEDUP
    if use_dedup:
        npad = NPAD_DEDUP
        base, rem = divmod(nu, NCORES)
        counts = [base + (1 if c < rem else 0) for c in range(NCORES)]
        starts = np.concatenate([[0], np.cumsum(counts)[:-1]])
        core_idx = [uniq[starts[c] : starts[c] + counts[c]] for c in range(NCORES)]
    else:
        npad = NPAD
        counts = list(_CORE_N)
        starts = list(_CORE_START)
        core_idx = [
            indices[starts[c] : starts[c] + counts[c]] for c in range(NCORES)
        ]

    # uniform valid count (dup-padded with row 0); -1 beyond it is trimmed
    # from the gather transfer by the SWDGE
    nvalid = min(-(-max(counts) // 16) * 16, npad)

    in_maps = []
    for c in range(NCORES):
        idx_pad = np.full(npad, -1, dtype=np.int16)
        idx_pad[: counts[c]] = core_idx[c]
        idx_pad[counts[c] : nvalid] = 0
        in_maps.append(
            {
                "whi": whi,
                "wlo": wlo,
                "xh": np.ascontiguousarray(xh),
                "xl": np.ascontiguousarray(xl),
                "xp": xp,
                "combm": combm,
                "idx": _wrap_idx(idx_pad),
            }
        )

    def assemble(results):
        cols = np.empty((B, sum(counts)), dtype=np.float32)
        for c in range(NCORES):
            cols[:, starts[c] : starts[c] + counts[c]] = results[c]["out"][
                :, : counts[c]
            ]
        if use_dedup:
            out = cols[:, inv]
        else:
            out = cols
        return np.ascontiguousarray(out.reshape(B, 1, R_TOTAL))

    return in_maps, assemble, npad, nvalid


def _filter_in_maps(nc, in_maps):
    names = set()
    from concourse import mybir

    for alloc in nc.m.functions[0].allocations:
        if isinstance(alloc, mybir.MemoryLocationSet) and alloc.kind == "ExternalInput":
            names.add(alloc.memorylocations[0].name)
    return [{k: v for k, v in m.items() if k in names} for m in in_maps]


def run_full(x, weight, indices, trace=False, mode="hi", dedup=True):
    """Returns (output, BassKernelResults)."""
    from concourse.bass_utils import run_bass_kernel_spmd

    if mode == "hi":
        in_maps, assemble, nvalid = _make_in_maps_hi(x, weight, indices)
        nc = _build_hi(nvalid)
        in_maps = _filter_in_maps(nc, in_maps)
        res = run_bass_kernel_spmd(nc, in_maps, list(range(NCORES)), trace=trace)
        return assemble(res.results), res

    in_maps, assemble, npad, nvalid = _make_in_maps(x, weight, indices, dedup=dedup)
    if mode == "fused2" and npad != NPAD_DEDUP:
        # the fused2 epilogue is only validated for the 512-wide dedup
        # layout; the rare >4096-unique fallback uses the fused tail
        mode = "fused"
    nc = _build(1, mode, False, npad, nvalid=nvalid)
    in_maps = _filter_in_maps(nc, in_maps)
    res = run_bass_kernel_spmd(nc, in_maps, list(range(NCORES)), trace=trace)
    return assemble(res.results), res


def kernel(x, weight, indices):
    out, _ = run_full(x, weight, indices)
    return out



# revision 24
# speedup vs baseline: 1.8810x; 1.0053x over previous
"""Trainium2 Bass kernel for gathered-row MLP decode matmul.

out[b, 0, r] = sum_d x[b, 0, d] * weight[indices[r], d]

Active path (kernel() -> run_full(mode="hi") -> _build_hi): dedup+sort the
indices on the host and shard them across 8 cores (~452 rows each). The
weight is cast to fp16 on the host (one 2-byte copy instead of an fp32 or
hi/lo pair: halves both HBM traffic and PE work; end-to-end scale-rel
error ~2.4e-4 against the 2e-2 gate). Each core gathers its rows in
128-row transpose-gather chunks that land matmul-ready [d%128, k, r];
the %128 tail chunk uses a non-transpose gather plus PE transposes that
hide under the other chunks' DMA transfers. Matmuls are weight-stationary
(lhsT = gathered [128, rows] tile, moving = x^T fp16 [128, 32] per
contraction tile), accumulating out^T [rows, 32] in PSUM; each chunk is
copied to SBUF and DMAed out transposed. The host transposes/assembles
per-core outputs and inverse-maps duplicates back to the original 4403
index order. The older fp32-accurate hi/lo modes are kept in _build()
for reference/fallback.
"""
import os
import sys
from contextlib import ExitStack

sys.path.insert(0, "/opt/trn_rl_repo")
os.environ.setdefault("MYCRO_LOCAL_CACHE", "1")

import numpy as np

D_FF = 11008
D_MODEL = 4096
R_TOTAL = 4403
B = 32
NCORES = 8
P = 128
KT = D_MODEL // P          # 32 contraction tiles
NPAD = 640                 # padded per-core index count (5*128), fallback
NPAD_DEDUP = 512           # padded per-core count for the dedup path
LO_SCALE = 2048.0          # wlo/xlo pre-scale (2^11)

# per-core share of the real 4403 indices (no-dedup fallback)
_CORE_N = [551, 551, 551, 550, 550, 550, 550, 550]
_CORE_START = [0]
for _n in _CORE_N[:-1]:
    _CORE_START.append(_CORE_START[-1] + _n)

_cache = {}


def _build(
    reps=1, mode="full", tiny_out=False, npad=NPAD, chunks=None, gbufs=2,
    nvalid=None,
):
    """mode: full (3-matmul) | fused (2-pass M=64 packing) | dma (gathers
    only) | dma_nt (non-transpose gathers) | mm (matmuls only).
    tiny_out: shrink the DRAM output to [B, 64] so bench-loop host
    transfers are negligible (timing only)."""
    if nvalid is None:
        nvalid = npad
    key = ("nc", reps, mode, tiny_out, npad, chunks, gbufs, nvalid)
    if key in _cache:
        return _cache[key]
    from concourse import bacc, mybir, tile

    f32 = mybir.dt.float32
    f16 = mybir.dt.float16
    i16 = mybir.dt.int16

    if chunks is None:
        chunks = tuple((i, min(256, npad - i)) for i in range(0, npad, 256))

    nc = bacc.Bacc(
        "TRN2", target_bir_lowering=False, debug=False, enable_asserts=False
    )
    whi_dram = nc.dram_tensor("whi", [D_FF, D_MODEL], f16, kind="ExternalInput").ap()
    wlo_dram = nc.dram_tensor("wlo", [D_FF, D_MODEL], f16, kind="ExternalInput").ap()
    if mode in ("fused", "fused2"):
        xp_dram = nc.dram_tensor("xp", [P, KT * 2 * B], f16, kind="ExternalInput").ap()
        if mode == "fused2":
            combm_dram = nc.dram_tensor(
                "combm", [P, B], f32, kind="ExternalInput"
            ).ap()
    else:
        xh_dram = nc.dram_tensor("xh", [P, KT * B], f16, kind="ExternalInput").ap()
        xl_dram = nc.dram_tensor("xl", [P, KT * B], f16, kind="ExternalInput").ap()
    idx_dram = nc.dram_tensor("idx", [P, npad // 16], i16, kind="ExternalInput").ap()
    out_cols = 64 if tiny_out else npad
    out_dram = nc.dram_tensor("out", [B, out_cols], f32, kind="ExternalOutput").ap()

    with tile.TileContext(nc) as tc, ExitStack() as ctx:
        consts = ctx.enter_context(tc.tile_pool(name="consts", bufs=1))
        whi_pool = ctx.enter_context(tc.tile_pool(name="whiT", bufs=gbufs))
        wlo_pool = ctx.enter_context(tc.tile_pool(name="wloT", bufs=gbufs))
        psum = ctx.enter_context(tc.tile_pool(name="psum", bufs=4, space="PSUM"))
        out_pool = ctx.enter_context(tc.tile_pool(name="outp", bufs=2))

        # idx first: the gathers (the critical path) depend only on it
        idx_sb = consts.tile([P, npad // 16], i16)
        nc.sync.dma_start(idx_sb[:], idx_dram)
        if mode in ("fused", "fused2"):
            xp_sb = consts.tile([P, KT * 2 * B], f16)
            nc.sync.dma_start(xp_sb[:], xp_dram)
        else:
            xh_sb = consts.tile([P, KT * B], f16)
            nc.sync.dma_start(xh_sb[:], xh_dram)
            xl_sb = consts.tile([P, KT * B], f16)
            nc.sync.dma_start(xl_sb[:], xl_dram)

        if mode == "mm":
            whiT_c = consts.tile([P, KT, 256], f16)
            nc.gpsimd.memset(whiT_c[:], 0.25)
            wloT_c = consts.tile([P, KT, 256], f16)
            nc.gpsimd.memset(wloT_c[:], 0.25)

        if mode == "fused2":
            # combM.T @ [hi; lo; 0] = hi + lo/2048 (host-built constant;
            # K padded to 128 so the fp32 matmul uses the proven full
            # partition-group shape)
            combM = consts.tile([P, B], f32)
            nc.sync.dma_start(combM[:], combm_dram)

        for _rep in range(reps):
            out_sb = out_pool.tile([B, npad], f32, tag="out_sb")
            if mode == "fused":
                t1_sb = out_pool.tile([64, npad], f32, tag="t1")
                outA_sb = out_pool.tile([B, npad], f32, tag="outA")

            for c, (r0, ncols) in enumerate(chunks):
                if mode in ("full", "fused", "fused2", "dma"):
                    # valid (non -1) indices in this chunk; the SWDGE trims
                    # the transfer to the valid prefix, so -1 tail padding
                    # costs no DMA.
                    nval_c = max(0, min(nvalid - r0, ncols))
                    # whiT[p, k, i] = whi[idx[r0+i], k*128 + p]
                    whiT = whi_pool.tile([P, KT, ncols], f16, tag="whiT")
                    wloT = wlo_pool.tile([P, KT, ncols], f16, tag="wloT")
                    if _rep == 0 and nval_c < ncols:
                        # first use of the slot: zero the never-gathered tail
                        # columns so downstream matmuls read finite data
                        nc.vector.memset(whiT[:, :, nval_c:], 0)
                        nc.vector.memset(wloT[:, :, nval_c:], 0)
                    nc.gpsimd.dma_gather(
                        out_ap=whiT[:],
                        in_ap=whi_dram,
                        idxs_ap=idx_sb[:, r0 // 16 : (r0 + ncols) // 16],
                        num_idxs=ncols,
                        num_idxs_reg=nval_c,
                        elem_size=D_MODEL,
                        transpose=True,
                    )
                    nc.gpsimd.dma_gather(
                        out_ap=wloT[:],
                        in_ap=wlo_dram,
                        idxs_ap=idx_sb[:, r0 // 16 : (r0 + ncols) // 16],
                        num_idxs=ncols,
                        num_idxs_reg=nval_c,
                        elem_size=D_MODEL,
                        transpose=True,
                    )
                elif mode == "dma_nt":
                    whiT = whi_pool.tile([P, -(-ncols // P), D_MODEL], f16, tag="whiT")
                    nc.gpsimd.dma_gather(
                        out_ap=whiT[:],
                        in_ap=whi_dram,
                        idxs_ap=idx_sb[:, r0 // 16 : (r0 + ncols) // 16],
                        num_idxs=ncols,
                        num_idxs_reg=ncols,
                        elem_size=D_MODEL,
                        transpose=False,
                    )
                    wloT = wlo_pool.tile([P, -(-ncols // P), D_MODEL], f16, tag="wloT")
                    nc.gpsimd.dma_gather(
                        out_ap=wloT[:],
                        in_ap=wlo_dram,
                        idxs_ap=idx_sb[:, r0 // 16 : (r0 + ncols) // 16],
                        num_idxs=ncols,
                        num_idxs_reg=ncols,
                        elem_size=D_MODEL,
                        transpose=False,
                    )
                else:
                    whiT = whiT_c
                    wloT = wloT_c

                if mode in ("dma", "dma_nt"):
                    continue

                if mode in ("fused", "fused2"):
                    # One PSUM chain: rows 0-31 accumulate xh*whi (hi chain);
                    # rows 32-63 accumulate xl_s*whi (mm1) AND xh*wlo_s (mm2).
                    # The group is opened by mm1@k=0 (spans rows 0-63) and
                    # closed by mm1@k=31, so mm2@k=31 is emitted before it.
                    psAB = psum.tile([64, ncols], mybir.dt.float32, tag="psA")

                    def mm1(k):
                        nc.tensor.matmul(
                            out=psAB[:],
                            lhsT=xp_sb[:, k * 2 * B : (k + 1) * 2 * B],
                            rhs=whiT[:, k, :],
                            start=(k == 0),
                            stop=(k == KT - 1),
                        )

                    def mm2(k):
                        nc.tensor.matmul(
                            out=psAB[B : 2 * B, :],
                            lhsT=xp_sb[:, k * 2 * B : k * 2 * B + B],
                            rhs=wloT[:, k, :],
                            start=False,
                            stop=False,
                        )

                    # mm1s first: they only depend on the whi gather, so the
                    # PE starts before wlo lands. mm1@KT-1 closes the group.
                    for k in range(KT - 1):
                        mm1(k)
                    for k in range(KT):
                        mm2(k)
                    mm1(KT - 1)
                    if mode == "fused2":
                        # recombine on the PE: out = combM.T @ [hi; lo; 0]
                        # = hi + lo/2048, landing directly on partitions 0-31
                        cmb_sb = out_pool.tile([P, ncols], f32, tag="cmb")
                        if _rep == 0 and c < 2:
                            nc.vector.memset(cmb_sb[2 * B :, :], 0)
                        nc.vector.tensor_copy(cmb_sb[: 2 * B, :], psAB[:])
                        psO = psum.tile([B, ncols], mybir.dt.float32, tag="psO")
                        nc.tensor.matmul(
                            out=psO[:], lhsT=combM[:], rhs=cmb_sb[:],
                            start=True, stop=True,
                        )
                        nc.scalar.copy(out_sb[:, r0 : r0 + ncols], psO[:])
                        if not tiny_out:
                            nc.sync.dma_start(
                                out_dram[:, r0 : r0 + ncols],
                                out_sb[:, r0 : r0 + ncols],
                            )
                        continue
                    # hi chain copied to partitions 0-31; lo-sum scaled on
                    # 32-63, realigned to 0-31 with an HWDGE SBUF-SBUF DMA
                    # (keeps the tail off the SWDGE queue the gathers use),
                    # then added and written out per chunk.
                    nc.scalar.copy(outA_sb[:, r0 : r0 + ncols], psAB[:B, :])
                    nc.vector.tensor_scalar_mul(
                        t1_sb[B : 2 * B, r0 : r0 + ncols],
                        psAB[B : 2 * B, :],
                        1.0 / LO_SCALE,
                    )
                    nc.sync.dma_start(
                        out_sb[:, r0 : r0 + ncols],
                        t1_sb[B : 2 * B, r0 : r0 + ncols],
                    )
                    nc.vector.tensor_add(
                        out_sb[:, r0 : r0 + ncols],
                        out_sb[:, r0 : r0 + ncols],
                        outA_sb[:, r0 : r0 + ncols],
                    )
                    if not tiny_out:
                        nc.sync.dma_start(
                            out_dram[:, r0 : r0 + ncols],
                            out_sb[:, r0 : r0 + ncols],
                        )
                    continue

                # mode full/mm: 3 matmul passes, both chains on partitions 0-31
                mcols = 256 if mode == "mm" else ncols
                psA = psum.tile([B, mcols], mybir.dt.float32, tag="psA")
                psB = psum.tile([B, mcols], mybir.dt.float32, tag="psB")
                for k in range(KT):
                    xh_k = xh_sb[:, k * B : (k + 1) * B]
                    xl_k = xl_sb[:, k * B : (k + 1) * B]
                    nc.tensor.matmul(
                        out=psA[:],
                        lhsT=xh_k,
                        rhs=whiT[:, k, :mcols],
                        start=(k == 0),
                        stop=(k == KT - 1),
                    )
                    nc.tensor.matmul(
                        out=psB[:],
                        lhsT=xh_k,
                        rhs=wloT[:, k, :mcols],
                        start=(k == 0),
                        stop=False,
                    )
                    nc.tensor.matmul(
                        out=psB[:],
                        lhsT=xl_k,
                        rhs=whiT[:, k, :mcols],
                        start=False,
                        stop=(k == KT - 1),
                    )
                dst = out_sb[:, r0 : r0 + ncols]
                nc.scalar.mul(dst, psB[:, :ncols], 1.0 / LO_SCALE)
                nc.vector.tensor_add(dst, dst, psA[:, :ncols])

            if mode == "fused":
                if tiny_out:
                    nc.sync.dma_start(out_dram, outA_sb[:, :out_cols])
                continue
            if mode == "fused2":
                continue
            if mode in ("dma", "dma_nt"):
                nc.vector.tensor_copy(out_sb[:, :64], whiT[:32, 0, :64])
            nc.sync.dma_start(out_dram, out_sb[:, :out_cols])

    nc.compile()
    _cache[key] = nc
    return nc


def _prep_scatter(nc, out_dram, obs, idx_sb, nv16, c):
    B_ = B
    nc.gpsimd.dma_scatter_add(
        out_ap=out_dram,
        in_ap=obs[:, c : c + 1, :],
        idxs_ap=idx_sb[:, nv16 + c * 8 : nv16 + c * 8 + 8],
        num_idxs=128,
        num_idxs_reg=128,
        elem_size=2 * B_,
        prepare_only=True,
        queue_num=1,
    )


def _build_hi(nvalid, gbufs=3, reps=1):
    """fp16-only weight-stationary kernel.

    One fp16 gather per 128-row chunk lands matmul-ready [d%128, k, r].
    Matmuls are weight-stationary: lhsT = gathered chunk [128, ncols<=128],
    rhs = x^T fp16 [128, 32] per contraction tile, accumulating
    psT[r, b] over the 32 k-tiles. Output is written transposed
    [nvalid, B]; the host transposes during assembly. fp16 on both sides
    gives ~2.4e-4 scale-rel error (gate is 2e-2).
    """
    key = ("hi5", nvalid, gbufs, reps)
    if key in _cache:
        return _cache[key]
    from concourse import bacc, mybir, tile

    f32 = mybir.dt.float32
    f16 = mybir.dt.float16
    i16 = mybir.dt.int16

    nfull = nvalid // 128      # full 128-row transpose-gather chunks
    nt = nvalid % 128          # odd-size tail chunk, non-transpose gather
    nch = nfull + (1 if nt else 0)
    nv16 = nvalid // 16

    nc = bacc.Bacc(
        "TRN2",
        target_bir_lowering=False,
        debug=False,
        enable_asserts=False,
        num_swdge_queues=2,
    )
    whi_dram = nc.dram_tensor("whi", [D_FF, D_MODEL], f16, kind="ExternalInput").ap()
    xh_dram = nc.dram_tensor("xh", [P, KT * B], f16, kind="ExternalInput").ap()
    if nt:
        id_dram = nc.dram_tensor("ident", [nt, nt], f16, kind="ExternalInput").ap()
    # first nv16 cols: gather indices; next nv16: iota rows for the scatter
    idx_dram = nc.dram_tensor("idx", [P, 2 * nv16], i16, kind="ExternalInput").ap()
    # 64-wide rows so the scatter elem is 256B (cols 32..63 are zero pad)
    out_dram = nc.dram_tensor("out", [nvalid, 2 * B], f32, kind="ExternalOutput").ap()

    with tile.TileContext(nc) as tc, ExitStack() as ctx:
        consts = ctx.enter_context(tc.tile_pool(name="consts", bufs=1))
        whi_pool = ctx.enter_context(tc.tile_pool(name="whiT", bufs=max(gbufs, nch)))
        psum = ctx.enter_context(tc.tile_pool(name="psum", bufs=4, space="PSUM"))

        idx_sb = consts.tile([P, 2 * nv16], i16)
        nc.sync.dma_start(idx_sb[:], idx_dram)
        xh_sb = consts.tile([P, KT * B], f16)
        nc.sync.dma_start(xh_sb[:], xh_dram)
        obs = consts.tile([P, nch, 2 * B], f32)
        nc.vector.memset(obs[:], 0)
        if nt:
            id_sb = consts.tile([nt, nt], f16)
            nc.sync.dma_start(id_sb[:], id_dram)
            # partitions >= nt are never written or read: no memset needed
            w4 = consts.tile([P, 1, D_MODEL], f16)

        # issue all gathers + output-scatter preps first so the Pool SEQ is
        # never blocked behind a trigger's data wait; desc-gen pipelines
        # ahead of the serialized DMA transfers
        whiTs = []
        if nt:
            # odd tail gathered FIRST (its transfer is the shortest and its
            # PE transposes hide under the remaining gathers' transfers)
            nc.gpsimd.dma_gather(
                out_ap=w4[:],
                in_ap=whi_dram,
                idxs_ap=idx_sb[:, nfull * 8 : nv16],
                num_idxs=nt,
                num_idxs_reg=nt,
                elem_size=D_MODEL,
                transpose=False,
            )
        for c in range(nfull):
            r0 = c * 128
            whiT = whi_pool.tile([P, KT, 128], f16, tag=f"whiT{c}")
            if c == nfull - 1:
                # last chunk in two half-row gathers: the k<16 matmuls can
                # start while the second half is still in flight
                for h in range(4):
                    nc.gpsimd.dma_gather(
                        out_ap=whiT[:, h * 8 : (h + 1) * 8, :],
                        in_ap=whi_dram[:, h * 1024 : (h + 1) * 1024],
                        idxs_ap=idx_sb[:, r0 // 16 : r0 // 16 + 8],
                        num_idxs=128,
                        num_idxs_reg=128,
                        elem_size=1024,
                        elem_step=D_MODEL,
                        transpose=True,
                    )
            else:
                nc.gpsimd.dma_gather(
                    out_ap=whiT[:],
                    in_ap=whi_dram,
                    idxs_ap=idx_sb[:, r0 // 16 : r0 // 16 + 8],
                    num_idxs=128,
                    num_idxs_reg=128,
                    elem_size=D_MODEL,
                    transpose=True,
                )
            whiTs.append(whiT)
        if nt:
            # on-chip transpose of the tail rows into matmul-ready layout
            whiT_nt = whi_pool.tile([P, KT, nt], f16, tag="whiTnt")
            for k in range(KT):
                psX = psum.tile([P, nt], f16, tag="psX")
                nc.tensor.transpose(
                    psX[:], w4[:nt, 0, k * P : (k + 1) * P], id_sb[:]
                )
                nc.vector.tensor_copy(whiT_nt[:, k, :], psX[:])
            whiTs.append(whiT_nt)

        order = list(range(nch))
        if nt and nch >= 2:
            # whiTs[nch-1] is the nt chunk (data ready early); run it before
            # the last full chunk, whose gather finishes last
            order = order[: nch - 2] + [nch - 1, nch - 2]
        for c in order:
            whiT = whiTs[c]
            rows = nt if (nt and c == nch - 1) else 128
            r0 = nfull * 128 if (nt and c == nch - 1) else c * 128
            psT = psum.tile([rows, B], f32, tag="psT")
            for k in range(KT):
                nc.tensor.matmul(
                    out=psT[:],
                    lhsT=whiT[:, k, :],
                    rhs=xh_sb[:, k * B : (k + 1) * B],
                    start=(k == 0),
                    stop=(k == KT - 1),
                )
            nc.scalar.copy(obs[:rows, c, :B], psT[:])
            nc.sync.dma_start(out_dram[r0 : r0 + rows, :B], obs[:rows, c, :B])

    nc.compile()
    _cache[key] = nc
    return nc


def _make_in_maps_hi(x, weight, indices):
    """Host prep for the hi kernel: dedup+shard indices, fp16 casts.

    Returns (in_maps, assemble_fn, nvalid)."""
    x = np.asarray(x, dtype=np.float32)
    weight = np.asarray(weight, dtype=np.float32)
    indices = np.asarray(indices, dtype=np.int64)

    whi = np.ascontiguousarray(weight.astype(np.float16))
    xt = np.ascontiguousarray(
        x[:, 0, :].reshape(B, KT, P).transpose(2, 1, 0).reshape(P, KT * B)
    )
    xh = np.ascontiguousarray(xt.astype(np.float16))

    uniq, inv = np.unique(indices, return_inverse=True)
    nu = len(uniq)
    base, rem = divmod(nu, NCORES)
    counts = [base + (1 if c < rem else 0) for c in range(NCORES)]
    starts = np.concatenate([[0], np.cumsum(counts)[:-1]])
    # multiples of 128 go through transpose gathers; the %16 tail is a
    # non-transpose gather + on-chip PE transpose
    nvalid = -(-max(counts) // 16) * 16

    iota = _wrap_idx(np.arange(nvalid, dtype=np.int16))
    ident = np.eye(nvalid % 128 or 1, dtype=np.float16)
    in_maps = []
    for c in range(NCORES):
        idx_pad = np.zeros(nvalid, dtype=np.int16)
        idx_pad[: counts[c]] = uniq[starts[c] : starts[c] + counts[c]]
        idx_full = np.concatenate([_wrap_idx(idx_pad), iota], axis=1)
        in_maps.append(
            {
                "whi": whi,
                "xh": xh,
                "idx": np.ascontiguousarray(idx_full),
                "ident": ident,
            }
        )

    def assemble(results):
        cols = np.empty((B, nu), dtype=np.float32)
        for c in range(NCORES):
            cols[:, starts[c] : starts[c] + counts[c]] = (
                results[c]["out"][: counts[c], :B].T
            )
        return np.ascontiguousarray(cols[:, inv].reshape(B, 1, R_TOTAL))

    return in_maps, assemble, nvalid


def _split_pair(a):
    """fp32 -> (hi fp16, (a-hi)*2048 fp16). hi + lo/2048 ~= a to ~2^-22 rel."""
    hi = a.astype(np.float16)
    lo = ((a - hi.astype(np.float32)) * LO_SCALE).astype(np.float16)
    return hi, lo


def _wrap_idx(idx_pad):
    """[npad] int16 -> [128, npad//16] wrapped-16 layout, replicated 8x."""
    npad = idx_pad.shape[0]
    blk = idx_pad.reshape(npad // 16, 16).T  # [16, npad//16]
    return np.ascontiguousarray(np.tile(blk, (8, 1)))


def _make_in_maps(x, weight, indices, dedup=True):
    """Returns (in_maps, assemble_fn, npad)."""
    x = np.asarray(x, dtype=np.float32)
    weight = np.ascontiguousarray(np.asarray(weight, dtype=np.float32))
    indices = np.asarray(indices, dtype=np.int64)

    whi, wlo = _split_pair(weight)
    whi = np.ascontiguousarray(whi)
    wlo = np.ascontiguousarray(wlo)

    # x^T staged so the DMA is contiguous: xt[p, k*32+b] = x[b, 0, k*128+p]
    xt = np.ascontiguousarray(
        x[:, 0, :].reshape(B, KT, P).transpose(2, 1, 0).reshape(P, KT * B)
    )
    xh, xl = _split_pair(xt)
    # packed [xh || xl] per contraction tile for the fused M=64 matmul
    xp = np.empty((P, KT, 2 * B), dtype=np.float16)
    xp[:, :, :B] = xh.reshape(P, KT, B)
    xp[:, :, B:] = xl.reshape(P, KT, B)
    xp = np.ascontiguousarray(xp.reshape(P, KT * 2 * B))
    combm = np.zeros((P, B), dtype=np.float32)
    combm[:B, :] = np.eye(B, dtype=np.float32)
    combm[B : 2 * B, :] = np.eye(B, dtype=np.float32) / LO_SCALE

    uniq, inv = np.unique(indices, return_inverse=True)
    nu = len(uniq)
    use_dedup = dedup and -(-nu // NCORES) <= NPAD_DEDUP
    if use_dedup:
        npad = NPAD_DEDUP
        base, rem = divmod(nu, NCORES)
        counts = [base + (1 if c < rem else 0) for c in range(NCORES)]
        starts = np.concatenate([[0], np.cumsum(counts)[:-1]])
        core_idx = [uniq[starts[c] : starts[c] + counts[c]] for c in range(NCORES)]
    else:
        npad = NPAD
        counts = list(_CORE_N)
        starts = list(_CORE_START)
        core_idx = [
            indices[starts[c] : starts[c] + counts[c]] for c in range(NCORES)
        ]

    # uniform valid count (dup-padded with row 0); -1 beyond it is trimmed
    # from the gather transfer by the SWDGE
    nvalid = min(-(-max(counts) // 16) * 16, npad)

    in_maps = []
    for c in range(NCORES):
        idx_pad = np.full(npad, -1, dtype=np.int16)
        idx_pad[: counts[c]] = core_idx[c]
        idx_pad[counts[c] : nvalid] = 0
        in_maps.append(
            {
                "whi": whi,
                "wlo": wlo,
                "xh": np.ascontiguousarray(xh),
                "xl": np.ascontiguousarray(xl),
                "xp": xp,
                "combm": combm,
                "idx": _wrap_idx(idx_pad),
            }
        )

    def assemble(results):
        cols = np.empty((B, sum(counts)), dtype=np.float32)
        for c in range(NCORES):
            cols[:, starts[c] : starts[c] + counts[c]] = results[c]["out"][
                :, : counts[c]
            ]
        if use_dedup:
            out = cols[:, inv]
        else:
            out = cols
        return np.ascontiguousarray(out.reshape(B, 1, R_TOTAL))

    return in_maps, assemble, npad, nvalid


def _filter_in_maps(nc, in_maps):
    names = set()
    from concourse import mybir

    for alloc in nc.m.functions[0].allocations:
        if isinstance(alloc, mybir.MemoryLocationSet) and alloc.kind == "ExternalInput":
            names.add(alloc.memorylocations[0].name)
    return [{k: v for k, v in m.items() if k in names} for m in in_maps]


def run_full(x, weight, indices, trace=False, mode="hi", dedup=True):
    """Returns (output, BassKernelResults)."""
    from concourse.bass_utils import run_bass_kernel_spmd

    if mode == "hi":
        in_maps, assemble, nvalid = _make_in_maps_hi(x, weight, indices)
        nc = _build_hi(nvalid)
        in_maps = _filter_in_maps(nc, in_maps)
        res = run_bass_kernel_spmd(nc, in_maps, list(range(NCORES)), trace=trace)
        return assemble(res.results), res

    in_maps, assemble, npad, nvalid = _make_in_maps(x, weight, indices, dedup=dedup)
    if mode == "fused2" and npad != NPAD_DEDUP:
        # the fused2 epilogue is only validated for the 512-wide dedup
        # layout; the rare >4096-unique fallback uses the fused tail
        mode = "fused"
    nc = _build(1, mode, False, npad, nvalid=nvalid)
    in_maps = _filter_in_maps(nc, in_maps)
    res = run_bass_kernel_spmd(nc, in_maps, list(range(NCORES)), trace=trace)
    return assemble(res.results), res


def kernel(x, weight, indices):
    out, _ = run_full(x, weight, indices)
    return out

